# revision 1
# baseline (speedup 1.0000x reference)
"""Trainium2 Bass kernel for nn_MoEDetector (moe_routing).

Strategy: data-parallel over batch B=8 -> one batch per NeuronCore.
Per-core program built around fp8e4m3 DoubleRow matmuls (K=256 contraction
per pass at 0.5 cycles/row -> 4x bf16 throughput on the PE):
  - router logits in fp32 (argmax-selection safe), group softmax ratios
  - GCN chain in single-term fp8: its output x2 is ~5e-4 of the residual
    stream, so fp8 quantization error there is negligible
  - 7 expert matmuls in 3-term fp8: X@W ~ X8@W8 + Xr@W8 + X8@Wr where
    X = X8 + Xr is an fp8 pair (residual capture ~1e-3, bf16-level) and
    32*W = W8 + Wr is a host-prepared scaled fp8 pair; exact gelu with
    the 1/32 fold in the activation scale
  - per-token top-1 selection folded into per-token coefficients
Host-side prep (layout/quantization only; all model FLOPs stay on device):
  - adjacency: degree-normalize, scale by 256, fp8-quantize, transpose
  - hidden states: fp8 pair (value + residual), transposed to [H, S]
  - the active len expert is determined by seq_lengths[b] (router masking
    forces the argmax), so each core gets only the active len weight and
    a 7-column router matrix
  - LN gain/bias folded into the syn expert weights
  - zero biases (the spec fills) are skipped; nonzero biases are
    supported via an extra K=1 rank-1 fp32 matmul accumulation step
"""

import numpy as np
import ml_dtypes
from contextlib import ExitStack

B, S, H = 8, 1024, 1536
THRESHOLD = 128
P = 128
ST = S // P          # 8 s-tiles
KT = H // P          # 12 h contraction tiles
TT = S // P          # 8 t-tiles for adjacency contraction
NCH = 512            # matmul moving free-dim chunk
NN = H // NCH        # 3 chunks of the H output dim
KD = KT // 2         # 6 DoubleRow passes over H
TD = TT // 2         # 4 DoubleRow passes over S
WS = 32.0            # host-side weight scale for fp8 range
ASC = 256.0          # host-side adjacency scale for fp8 range
EPS = 1e-5

_BF16 = ml_dtypes.bfloat16
_F8 = ml_dtypes.float8_e4m3

_prog_cache = {}


def _build_program(cfg):
    """cfg = (router_bias_nz, syn_bias_nz, len_bias_nz, sem_bias_nz, cls_bias_nz)"""
    import concourse.bass as bass
    import concourse.tile as tile
    from concourse import bacc, masks, mybir

    rb_nz, synb_nz, lenb_nz, semb_nz, clsb_nz = cfg
    f32 = mybir.dt.float32
    i32 = mybir.dt.int32
    bf16 = mybir.dt.bfloat16
    fp8 = mybir.dt.float8e4
    AF = mybir.ActivationFunctionType
    ALU = mybir.AluOpType
    AX = mybir.AxisListType
    DR = mybir.MatmulPerfMode.DoubleRow
    ts = bass.ts

    nc = bacc.Bacc("TRN2", target_bir_lowering=False, debug=False)

    # ---- DRAM I/O ----
    hsb_d = nc.dram_tensor("hsb", [S, H], bf16, kind="ExternalInput").ap()
    hb1T_d = nc.dram_tensor("hb1T", [H, S], bf16, kind="ExternalInput").ap()
    hb2T_d = nc.dram_tensor("hb2T", [H, S], bf16, kind="ExternalInput").ap()
    hb3T_d = nc.dram_tensor("hb3T", [H, S], bf16, kind="ExternalInput").ap()
    hs8T_d = nc.dram_tensor("hs8T", [H, S], fp8, kind="ExternalInput").ap()
    hsrT_d = nc.dram_tensor("hsrT", [H, S], fp8, kind="ExternalInput").ap()
    adjT_d = nc.dram_tensor("adjT", [S, S], fp8, kind="ExternalInput").ap()
    rw1_d = nc.dram_tensor("rw1", [H, 7], bf16, kind="ExternalInput").ap()
    rw2_d = nc.dram_tensor("rw2", [H, 7], bf16, kind="ExternalInput").ap()
    wg1_d = nc.dram_tensor("wg1", [H, H], fp8, kind="ExternalInput").ap()
    wg2_d = nc.dram_tensor("wg2", [H, H], fp8, kind="ExternalInput").ap()
    wexp_d = []  # (w8, wr) per expert: len, sem0-2, syn0-2
    for nm in ["len", "sem0", "sem1", "sem2", "syn0", "syn1", "syn2"]:
        wexp_d.append((
            nc.dram_tensor(f"w{nm}8", [H, H], fp8, kind="ExternalInput").ap(),
            nc.dram_tensor(f"w{nm}r", [H, H], fp8, kind="ExternalInput").ap(),
        ))
    wcls_d = nc.dram_tensor("wcls", [H, 2], bf16, kind="ExternalInput").ap()
    br_d = nc.dram_tensor("br", [1, 7], f32, kind="ExternalInput").ap() if rb_nz else None
    bsyn_d = nc.dram_tensor("bsyn", [3, H], f32, kind="ExternalInput").ap() if synb_nz else None
    blen_d = nc.dram_tensor("blen", [1, H], f32, kind="ExternalInput").ap() if lenb_nz else None
    bsem_d = nc.dram_tensor("bsem", [3, H], f32, kind="ExternalInput").ap() if semb_nz else None
    bcls_d = nc.dram_tensor("bcls", [1, 2], f32, kind="ExternalInput").ap() if clsb_nz else None
    out_d = nc.dram_tensor("out", [P, ST, 2], f32, kind="ExternalOutput").ap()

    hs_r = hsb_d.rearrange("(a p) h -> p a h", p=P)
    hb1T_r = hb1T_d.rearrange("(k p) s -> p k s", p=P)
    hb2T_r = hb2T_d.rearrange("(k p) s -> p k s", p=P)
    hb3T_r = hb3T_d.rearrange("(k p) s -> p k s", p=P)
    hs8T_r = hs8T_d.rearrange("(k p) s -> p k s", p=P)
    hsrT_r = hsrT_d.rearrange("(k p) s -> p k s", p=P)
    adjT_r = adjT_d.rearrange("(t p) s -> p t s", p=P)
    rw1_r = rw1_d.rearrange("(k p) e -> p k e", p=P)
    rw2_r = rw2_d.rearrange("(k p) e -> p k e", p=P)
    wcls_r = wcls_d.rearrange("(k p) c -> p k c", p=P)
    out_r = out_d

    with tile.TileContext(nc) as tc, ExitStack() as ctx:
        # ---- pools ----
        const = ctx.enter_context(tc.tile_pool(name="const", bufs=1))
        hspool = ctx.enter_context(tc.tile_pool(name="hspool", bufs=1))
        f8pool = ctx.enter_context(tc.tile_pool(name="f8pool", bufs=1))
        wpool = ctx.enter_context(tc.tile_pool(name="wpool", bufs=2))
        stage = ctx.enter_context(tc.tile_pool(name="stage", bufs=2))
        small = ctx.enter_context(tc.tile_pool(name="small", bufs=2))
        acc = ctx.enter_context(tc.tile_pool(name="acc", bufs=4, space="PSUM"))
        spsum = ctx.enter_context(tc.tile_pool(name="spsum", bufs=2, space="PSUM"))

        # ---- constants (gpsimd DMA queue, parallel to sync queue) ----
        id_f32 = const.tile([P, P], f32, tag="idf")
        masks.make_identity(nc, id_f32[:])
        id_bf = const.tile([P, P], bf16, tag="idb")
        masks.make_identity(nc, id_bf[:])
        rw1_sb = const.tile([P, KT, 7], bf16, tag="rw1")
        nc.gpsimd.dma_start(rw1_sb[:], rw1_r)
        rw2_sb = const.tile([P, KT, 7], bf16, tag="rw2")
        nc.gpsimd.dma_start(rw2_sb[:], rw2_r)
        wcls_sb = const.tile([P, KT, 2], bf16, tag="wcls")
        nc.gpsimd.dma_start(wcls_sb[:], wcls_r)
        eps_t = const.tile([P, 1], f32, tag="eps")
        nc.vector.memset(eps_t[:], EPS)
        ones_row = None
        if any(x is not None for x in (br_d, bsyn_d, blen_d, bsem_d, bcls_d)):
            ones_row = const.tile([1, P], f32, tag="ones")
            nc.vector.memset(ones_row[:], 1.0)

        def bias_row(dram_ap, n, tag):
            t = const.tile([1, n], f32, tag=tag)
            nc.gpsimd.dma_start(t[:], dram_ap)
            return t

        br_sb = bias_row(br_d, 7, "br") if br_d is not None else None
        blen_sb = bias_row(blen_d, H, "blen") if blen_d is not None else None
        bsem_sb = ([bias_row(bsem_d[e : e + 1, :], H, f"bsem{e}") for e in range(3)]
                   if bsem_d is not None else None)
        bsyn_sb = ([bias_row(bsyn_d[e : e + 1, :], H, f"bsyn{e}") for e in range(3)]
                   if bsyn_d is not None else None)

        # ---- persistent SBUF tensors ----
        hs_all = hspool.tile([P, ST, H], bf16, tag="hs")      # hs -> resid -> fused
        hs8T = f8pool.tile([P, KT, S], fp8, tag="hs8T")
        hsrT = f8pool.tile([P, KT, S], fp8, tag="hsrT")
        adjT = f8pool.tile([P, TT, S], fp8, tag="adjT")       # 256 * Anorm^T

        # ---- DMA issue order on the sync queue (sets arrival times) ----
        def load_w(wdram, tag):
            wt = wpool.tile([P, KT, H], fp8, tag=tag)
            nc.sync.dma_start(wt[:], wdram.rearrange("(k p) d -> p k d", p=P))
            return wt

        w_g1 = wpool.tile([P, KT, H], fp8, tag="w8")
        wg1_r = wg1_d.rearrange("(k p) d -> p k d", p=P)
        nc.sync.dma_start(w_g1[:, 0:6, :], wg1_r[:, 0:6, :])
        nc.sync.dma_start(hs8T[:, 0:6, :], hs8T_r[:, 0:6, :])
        nc.sync.dma_start(w_g1[:, 6:12, :], wg1_r[:, 6:12, :])
        nc.sync.dma_start(hs8T[:, 6:12, :], hs8T_r[:, 6:12, :])
        nc.sync.dma_start(adjT[:], adjT_r)
        w_g2 = load_w(wg2_d, "w8")
        hb1T = wpool.tile([P, KT, S], bf16, tag="w8")
        nc.sync.dma_start(hb1T[:], hb1T_r)
        hb2T = wpool.tile([P, KT, S], bf16, tag="wr")
        nc.sync.dma_start(hb2T[:], hb2T_r)
        hb3T = wpool.tile([P, KT, S], bf16, tag="wr")
        nc.sync.dma_start(hb3T[:], hb3T_r)
        nc.sync.dma_start(hs_all[:], hs_r)
        nc.sync.dma_start(hsrT[:], hsrT_r)
        wexp_sb = [(load_w(w8d, "w8"), load_w(wrd, "wr")) for w8d, wrd in wexp_d]

        # ---- GCN: S1 = hs8 @ W1q (fp8 DR), evict /32 -> fp8 [s, d] ----
        s_sb = f8pool.tile([P, ST, H], fp8, tag="s12")
        for m in range(ST):
            for n in range(NN):
                ps = acc.tile([P, NCH], f32, tag="acc")
                for j in range(KD):
                    nc.tensor.matmul(ps[:], hs8T[:, 2 * j : 2 * j + 2, ts(m, P)],
                                     w_g1[:, 2 * j : 2 * j + 2, ts(n, NCH)],
                                     start=(j == 0), stop=(j == KD - 1), perf_mode=DR)
                nc.scalar.activation(s_sb[:, m, ts(n, NCH)], ps[:], AF.Copy,
                                     scale=1.0 / WS)

        # ---- x1T = relu(Anorm @ S1)^T via lhsT=S1: psum = ASC*x1pre ----
        # store 32*relu(x1) in fp8
        x1T = f8pool.tile([P, KT, S], fp8, tag="x1T")
        for dt_i in range(KT):
            for sc in range(2):
                ps = acc.tile([P, NCH], f32, tag="acc")
                for j in range(TD):
                    nc.tensor.matmul(ps[:], s_sb[:, 2 * j : 2 * j + 2, ts(dt_i, P)],
                                     adjT[:, 2 * j : 2 * j + 2, ts(sc, NCH)],
                                     start=(j == 0), stop=(j == TD - 1), perf_mode=DR)
                nc.scalar.activation(x1T[:, dt_i, ts(sc, NCH)], ps[:], AF.Relu,
                                     scale=WS / ASC)

        # ---- router: fp32-exact logits from bf16 triple/pair split ----
        # hs = hb1+hb2+hb3, rw = rw1+rw2 (bf16 splits, host-prepared).
        # logits = hb1@rw1 + hb1@rw2 + hb2@rw1 + hb2@rw2 + hb3@rw1; the
        # dropped terms are O(1e-8) so argmax matches fp32 exactly.
        logit = small.tile([P, ST, 7], f32, tag="logit", bufs=1)
        nc.vector.memset(logit[:], 0.0)
        terms = ((hb1T, rw1_sb), (hb1T, rw2_sb), (hb2T, rw1_sb),
                 (hb2T, rw2_sb), (hb3T, rw1_sb))
        for k in range(KT):
            rlog = spsum.tile([P, ST, 7], f32, tag="sp")
            for m in range(ST):
                for t_i, (hb, rwt) in enumerate(terms):
                    nc.tensor.matmul(rlog[:, m, :], hb[:, k, ts(m, P)],
                                     rwt[:, k, :], start=(t_i == 0),
                                     stop=(t_i == len(terms) - 1))
            nc.vector.tensor_add(logit[:], logit[:], rlog[:])
        if br_sb is not None:
            rlog = spsum.tile([P, ST, 7], f32, tag="sp")
            for m in range(ST):
                nc.tensor.matmul(rlog[:, m, :], ones_row[:], br_sb[:],
                                 start=True, stop=True)
            nc.vector.tensor_add(logit[:], logit[:], rlog[:])

        # ---- router math: group softmax ratios + top-1 coefficients ----
        # logits are O(1): exp() without max-subtraction is safe, and softmax
        # ratios are shift-invariant so this matches the reference exactly.
        e_sb = small.tile([P, ST, 7], f32, tag="esb")
        nc.scalar.activation(e_sb[:], logit[:], AF.Exp)
        syn_e = small.tile([P, ST], f32, tag="syn_e")
        nc.vector.tensor_reduce(syn_e[:], e_sb[:, :, 0:3], axis=AX.X, op=ALU.max)
        sem_e = small.tile([P, ST], f32, tag="sem_e")
        nc.vector.tensor_reduce(sem_e[:], e_sb[:, :, 4:7], axis=AX.X, op=ALU.max)
        rden = small.tile([P, ST], f32, tag="rden")
        nc.vector.tensor_add(rden[:], syn_e[:], sem_e[:])
        nc.vector.tensor_add(rden[:], rden[:], e_sb[:, :, 3])
        nc.vector.reciprocal(rden[:], rden[:])

        csyn = small.tile([P, ST, 3], f32, tag="csyn")
        csem = small.tile([P, ST, 3], f32, tag="csem")
        clen = small.tile([P, ST], f32, tag="clen")
        nc.vector.tensor_mul(clen[:], e_sb[:, :, 3], rden[:])

        def group_coefs(cout, base, w_e):
            """cout[:,:,e] = rden * w_e * mask_e; first-max argmax over logit
            columns base..base+2 (matches jnp.argmax tie-breaking)."""
            l0, l1, l2 = (logit[:, :, base + i] for i in range(3))
            s0 = small.tile([P, ST], f32, tag="s0")
            ge02 = small.tile([P, ST], f32, tag="ge02")
            nc.vector.tensor_tensor(out=s0[:], in0=l0, in1=l1, op=ALU.is_ge)
            nc.vector.tensor_tensor(out=ge02[:], in0=l0, in1=l2, op=ALU.is_ge)
            nc.vector.tensor_mul(s0[:], s0[:], ge02[:])
            s1 = small.tile([P, ST], f32, tag="s1")
            ge12 = small.tile([P, ST], f32, tag="ge12")
            nc.vector.tensor_tensor(out=ge12[:], in0=l1, in1=l2, op=ALU.is_ge)
            nc.vector.tensor_mul(s1[:], s0[:], ge12[:])
            nc.vector.tensor_tensor(out=s1[:], in0=ge12[:], in1=s1[:], op=ALU.subtract)
            s2 = small.tile([P, ST], f32, tag="s2")
            nc.vector.tensor_add(s2[:], s0[:], s1[:])
            nc.vector.tensor_scalar(out=s2[:], in0=s2[:], scalar1=-1.0, scalar2=1.0,
                                    op0=ALU.mult, op1=ALU.add)
            for e, sm in enumerate((s0, s1, s2)):
                nc.vector.tensor_mul(cout[:, :, e], sm[:], w_e)
                nc.vector.tensor_mul(cout[:, :, e], cout[:, :, e], rden[:])

        group_coefs(csyn, 0, syn_e[:])
        group_coefs(csem, 4, sem_e[:])

        # ---- S2 = (32 x1) @ W2q: psum = 32*32*S2pre; store 32*S2 in fp8 ----
        s2_sb = f8pool.tile([P, ST, H], fp8, tag="s12")
        for m in range(ST):
            for n in range(NN):
                ps = acc.tile([P, NCH], f32, tag="acc")
                for j in range(KD):
                    nc.tensor.matmul(ps[:], x1T[:, 2 * j : 2 * j + 2, ts(m, P)],
                                     w_g2[:, 2 * j : 2 * j + 2, ts(n, NCH)],
                                     start=(j == 0), stop=(j == KD - 1), perf_mode=DR)
                nc.scalar.activation(s2_sb[:, m, ts(n, NCH)], ps[:], AF.Copy,
                                     scale=1.0 / WS)

        # ---- residual + LayerNorm -> sh (bf16), interleaved with experts ----
        sh_t = [None] * ST

        def do_ln(m):
            stats = small.tile([P, NN, 6], f32, tag="stats")
            for c in range(NN):
                nc.vector.bn_stats(stats[:, c, :], hs_all[:, m, ts(c, NCH)])
            mv = small.tile([P, 2], f32, tag="mv")
            nc.vector.bn_aggr(mv[:], stats[:])
            # rstd = rsqrt(var + eps) via bit-trick seed + 2 Newton steps on
            # DVE -- keeps Sqrt off the Act engine so the expert-phase stays
            # on one activation table (gelu/copy/identity).
            veps = small.tile([P, 1], f32, tag="veps")
            nc.vector.tensor_scalar(out=veps[:], in0=mv[:, 1:2], scalar1=EPS,
                                    scalar2=None, op0=ALU.add)
            rsd_i = small.tile([P, 1], i32, tag="rsdi")
            nc.vector.tensor_scalar(out=rsd_i[:], in0=veps[:].bitcast(i32),
                                    scalar1=1, scalar2=None,
                                    op0=ALU.logical_shift_right)
            nc.vector.tensor_scalar(out=rsd_i[:], in0=rsd_i[:], scalar1=-1,
                                    scalar2=0x5F3759DF, op0=ALU.mult, op1=ALU.add)
            rstd = rsd_i[:].bitcast(f32)
            nwt = small.tile([P, 1], f32, tag="nwt")
            for _ in range(1):
                nc.vector.tensor_mul(nwt[:], rstd, rstd)
                nc.vector.tensor_mul(nwt[:], nwt[:], veps[:])
                nc.vector.tensor_scalar(out=nwt[:], in0=nwt[:], scalar1=-0.5,
                                        scalar2=1.5, op0=ALU.mult, op1=ALU.add)
                nc.vector.tensor_mul(rstd, rstd, nwt[:])
            nmr = small.tile([P, 1], f32, tag="nmr")
            nc.vector.tensor_mul(nmr[:], mv[:, 0:1], rstd)
            nc.vector.tensor_scalar_mul(nmr[:], nmr[:], -1.0)
            sh = stage.tile([P, H], bf16, tag="shm", bufs=3)
            nc.scalar.activation(sh[:], hs_all[:, m, :], AF.Identity,
                                 bias=nmr[:], scale=rstd)
            sh_t[m] = sh


        # ---- x2: psum = ASC*32*x2pre; resid += relu(psum)/8192 ----
        for m in range(ST):
            for n in range(NN):
                ps = acc.tile([P, NCH], f32, tag="acc")
                for j in range(TD):
                    nc.tensor.matmul(ps[:], adjT[:, 2 * j : 2 * j + 2, ts(m, P)],
                                     s2_sb[:, 2 * j : 2 * j + 2, ts(n, NCH)],
                                     start=(j == 0), stop=(j == TD - 1), perf_mode=DR)
                g = stage.tile([P, NCH], f32, tag="hTf", bufs=3)
                nc.scalar.activation(g[:], ps[:], AF.Relu, scale=1.0 / (ASC * WS))
                eng = nc.gpsimd if n == 2 else nc.vector
                eng.tensor_add(hs_all[:, m, ts(n, NCH)],
                               hs_all[:, m, ts(n, NCH)], g[:])


        shared8T = f8pool.tile([P, KT, S], fp8, tag="x1T")   # reuse x1T slot
        sharedrT = f8pool.tile([P, KT, S], fp8, tag="s12")   # reuse S slot

        def do_shT(m):
            shT_bf = stage.tile([P, KT, P], bf16, tag="fuT", bufs=3)
            nc.scalar.dma_start_transpose(shT_bf[:], sh_t[m][:])
            nc.scalar.activation(shared8T[:, :, ts(m, P)], shT_bf[:], AF.Copy)
            nc.gpsimd.tensor_tensor(out=sharedrT[:, :, ts(m, P)], in0=shT_bf[:],
                                    in1=shared8T[:, :, ts(m, P)], op=ALU.subtract)

        # ---- experts: 3-term fp8 DR, weighted top-1 accumulation into hs_all ----
        def expert_mm(ei, x8, xr, w8, wr, coef, bias_sb, after_row=None):
            for m in range(ST):
                for n in range(NN):
                    ps = acc.tile([P, NCH], f32, tag="acc")
                    for t_i, (xx, ww) in enumerate(((x8, w8), (xr, w8), (x8, wr))):
                        for j in range(KD):
                            last = (t_i == 2 and j == KD - 1 and bias_sb is None)
                            nc.tensor.matmul(
                                ps[:], xx[:, 2 * j : 2 * j + 2, ts(m, P)],
                                ww[:, 2 * j : 2 * j + 2, ts(n, NCH)],
                                start=(t_i == 0 and j == 0), stop=last, perf_mode=DR)
                    if bias_sb is not None:
                        nc.tensor.matmul(ps[:], ones_row[:], bias_sb[:, ts(n, NCH)],
                                         start=False, stop=True)
                    g = stage.tile([P, NCH], f32, tag="hTf", bufs=3)
                    nc.scalar.activation(g[:], ps[:], AF.Gelu, scale=1.0 / WS)
                    dst = hs_all[:, m, ts(n, NCH)]
                    if ei == 0:
                        nc.vector.tensor_scalar_mul(dst, g[:], coef[:, m : m + 1])
                    else:
                        nc.vector.scalar_tensor_tensor(
                            out=dst, in0=g[:], scalar=coef[:, m : m + 1], in1=dst,
                            op0=ALU.mult, op1=ALU.add)
                if after_row is not None:
                    after_row(m)

        for m in range(3):
            do_ln(m)

        # len expert first: its matmuls cover the LN -> sharedT latency; the
        # per-row hook drains the sharedT transposes and the remaining LNs.
        def len_after(m):
            do_shT(m)
            if m + 3 < ST:
                do_ln(m + 3)

        expert_mm(0, hs8T, hsrT, wexp_sb[0][0], wexp_sb[0][1], clen[:, :], blen_sb,
                  after_row=len_after)
        for e in range(3):  # sem experts on hs
            expert_mm(1 + e, hs8T, hsrT, wexp_sb[1 + e][0], wexp_sb[1 + e][1],
                      csem[:, :, e], bsem_sb[e] if bsem_sb else None)
        for e in range(2):  # syn experts 0,1 on shared
            expert_mm(4 + e, shared8T, sharedrT, wexp_sb[4 + e][0], wexp_sb[4 + e][1],
                      csyn[:, :, e], bsyn_sb[e] if bsyn_sb else None)

        # ---- last syn expert with the fusedT + cls tail interleaved per row ----
        bcls_sb = bias_row(bcls_d, 2, "bcls") if bcls_d is not None else None
        out_sb = small.tile([P, ST, 2], f32, tag="outsb", bufs=1)

        fuT_t = [None] * ST

        def fused_pre(m):
            fuT = stage.tile([P, KT, P], bf16, tag="fuT", bufs=3)
            nc.scalar.dma_start_transpose(fuT[:], hs_all[:, m, :])
            fuT_t[m] = fuT

        def fused_cls(m):
            fuT = fuT_t[m]
            cps = spsum.tile([P, 2], f32, tag="cls")
            for k in range(KT):
                last = (k == KT - 1) and (bcls_sb is None)
                nc.tensor.matmul(cps[:], fuT[:, k, :], wcls_sb[:, k, :],
                                 start=(k == 0), stop=last)
            if bcls_sb is not None:
                nc.tensor.matmul(cps[:], ones_row[:], bcls_sb[:],
                                 start=False, stop=True)
            nc.vector.tensor_copy(out_sb[:, m, :], cps[:])

        def syn2_row(m):
            if m > 0:
                fused_pre(m - 1)
            if m > 1:
                fused_cls(m - 2)

        expert_mm(6, shared8T, sharedrT, wexp_sb[6][0], wexp_sb[6][1],
                  csyn[:, :, 2], bsyn_sb[2] if bsyn_sb else None,
                  after_row=syn2_row)
        fused_cls(ST - 2)
        fused_pre(ST - 1)
        fused_cls(ST - 1)
        nc.sync.dma_start(out_r, out_sb[:])

    nc.compile()
    return nc


def _get_program(cfg):
    if cfg not in _prog_cache:
        _prog_cache[cfg] = _build_program(cfg)
    return _prog_cache[cfg]


def _fp8_pair(w):
    """w -> (q8(32w), q8(32w - float(q8(32w)))) as contiguous fp8 arrays."""
    ws = (WS * w).astype(np.float32)
    w8 = ws.astype(_F8)
    wr = (ws - w8.astype(np.float32)).astype(_F8)
    return np.ascontiguousarray(w8), np.ascontiguousarray(wr)


def kernel(**inputs):
    from concourse import bass_utils

    hs = np.asarray(inputs["hidden_states"], dtype=np.float32)
    adj = np.asarray(inputs["adj_matrix"], dtype=np.float32)
    seq_lengths = np.asarray(inputs["seq_lengths"])
    router_w = np.asarray(inputs["router_w"], dtype=np.float32)
    router_b = np.asarray(inputs["router_b"], dtype=np.float32)
    gcn1_w = np.asarray(inputs["gcn1_w"], dtype=np.float32)
    gcn2_w = np.asarray(inputs["gcn2_w"], dtype=np.float32)
    ln_g = np.asarray(inputs["ln_g"], dtype=np.float32)
    ln_b = np.asarray(inputs["ln_b"], dtype=np.float32)
    syn_w = np.asarray(inputs["syn_w"], dtype=np.float32)
    syn_b = np.asarray(inputs["syn_b"], dtype=np.float32)
    len_short_w = np.asarray(inputs["len_short_w"], dtype=np.float32)
    len_short_b = np.asarray(inputs["len_short_b"], dtype=np.float32)
    len_long_w = np.asarray(inputs["len_long_w"], dtype=np.float32)
    len_long_b = np.asarray(inputs["len_long_b"], dtype=np.float32)
    sem_w = np.asarray(inputs["sem_w"], dtype=np.float32)
    sem_b = np.asarray(inputs["sem_b"], dtype=np.float32)
    cls_w = np.asarray(inputs["cls_w"], dtype=np.float32)
    cls_b = np.asarray(inputs["cls_b"], dtype=np.float32)

    # fold LN affine into syn expert weights: (x*g + b) @ W = x @ (g[:,None]*W) + b@W
    syn_w_f = (ln_g[None, :, None] * syn_w).astype(np.float32)
    syn_b_f = (syn_b + np.einsum("h,ehd->ed", ln_b, syn_w)).astype(np.float32)

    is_short = seq_lengths <= THRESHOLD

    cfg = (
        bool(np.any(router_b != 0)),
        bool(np.any(syn_b_f != 0)),
        bool(np.any(len_short_b != 0) or np.any(len_long_b != 0)),
        bool(np.any(sem_b != 0)),
        bool(np.any(cls_b != 0)),
    )
    nc = _get_program(cfg)

    wg1_8, _ = _fp8_pair(gcn1_w)
    wg2_8, _ = _fp8_pair(gcn2_w)
    wls = _fp8_pair(len_short_w)
    wll = _fp8_pair(len_long_w)
    wsem = [_fp8_pair(sem_w[e]) for e in range(3)]
    wsyn = [_fp8_pair(syn_w_f[e]) for e in range(3)]
    wcls = np.ascontiguousarray(cls_w.astype(_BF16))

    in_maps = []
    for b in range(B):
        lencol = 3 if is_short[b] else 4
        rw7 = np.ascontiguousarray(np.concatenate(
            [router_w[:, 0:3], router_w[:, lencol : lencol + 1], router_w[:, 5:8]],
            axis=1, dtype=np.float32))
        wlen = wls if is_short[b] else wll
        hsb = hs[b]
        hs8 = hsb.astype(_F8)
        hsr = (hsb - hs8.astype(np.float32)).astype(_F8)
        hb1 = hsb.astype(_BF16)
        r = hsb - hb1.astype(np.float32)
        hb2 = r.astype(_BF16)
        hb3 = (r - hb2.astype(np.float32)).astype(_BF16)
        rw1 = rw7.astype(_BF16)
        rw2 = (rw7 - rw1.astype(np.float32)).astype(_BF16)
        deg = np.clip(adj[b].sum(axis=1, keepdims=True), 1e-9, None)
        adjq = (ASC * adj[b] / deg).astype(_F8)
        m = {
            "hsb": np.ascontiguousarray(hb1),
            "hb1T": np.ascontiguousarray(hb1.T),
            "hb2T": np.ascontiguousarray(hb2.T),
            "hb3T": np.ascontiguousarray(hb3.T),
            "hs8T": np.ascontiguousarray(hs8.T),
            "hsrT": np.ascontiguousarray(hsr.T),
            "adjT": np.ascontiguousarray(adjq.T),
            "rw1": np.ascontiguousarray(rw1),
            "rw2": np.ascontiguousarray(rw2),
            "wg1": wg1_8, "wg2": wg2_8,
            "wlen8": wlen[0], "wlenr": wlen[1],
            "wcls": wcls,
        }
        for e in range(3):
            m[f"wsem{e}8"], m[f"wsem{e}r"] = wsem[e]
            m[f"wsyn{e}8"], m[f"wsyn{e}r"] = wsyn[e]
        if cfg[0]:
            br7 = np.concatenate(
                [router_b[0:3], router_b[lencol : lencol + 1], router_b[5:8]])
            m["br"] = br7.reshape(1, 7).astype(np.float32)
        if cfg[1]:
            m["bsyn"] = (WS * syn_b_f).astype(np.float32)
        if cfg[2]:
            m["blen"] = (WS * (len_short_b if is_short[b]
                               else len_long_b)).reshape(1, H).astype(np.float32)
        if cfg[3]:
            m["bsem"] = (WS * sem_b).astype(np.float32)
        if cfg[4]:
            m["bcls"] = cls_b.reshape(1, 2).astype(np.float32)
        in_maps.append(m)

    try:
        res = bass_utils.run_bass_kernel_spmd(nc, in_maps, core_ids=list(range(B)))
    except Exception:
        # transient device wedge (NRT_EXEC_UNIT_UNRECOVERABLE) clears on retry
        res = bass_utils.run_bass_kernel_spmd(nc, in_maps, core_ids=list(range(B)))
    globals()["_last_results"] = res
    out = np.stack([
        np.asarray(res.results[b]["out"], dtype=np.float32)
        .transpose(1, 0, 2).reshape(S, 2)
        for b in range(B)
    ])
    return out



# revision 23
# speedup vs baseline: 1.0342x; 1.0342x over previous
"""Trainium2 Bass kernel for nn_MoEDetector (moe_routing).

Strategy: data-parallel over batch B=8 -> one batch per NeuronCore, with
top-1 sparse expert dispatch on-device:
  - router logits in fp32 (argmax-selection safe), group softmax ratios
  - GCN chain in single-term fp8 (output is ~5e-4 of the residual stream)
  - tokens are counting-sorted by their syn/sem argmax class on device
    (sparse_gather) into 3 fixed 512-token capacity regions per group;
    dma_gather (indirect DMA) fetches each region's tokens from DRAM in
    transposed [h, slot] layout at zero PE cost
  - each region statically maps to one expert, so the region matmuls run
    at full fp8 DoubleRow speed with 3-term splits (X@W ~ X8@W8 + Xr@W8 +
    X8@Wr); 24 region-tile evals replace 48 dense sem+syn evals
  - the len expert choice is forced per-batch by seq_lengths via router
    masking, so it runs dense in original token order
  - cls head is linear: three partial outputs (len / sem / syn order) are
    produced separately and summed on host after unpermuting
  - expert weights and router hb-splits stream through SBUF in 512-column
    chunks (n-outer loops) to fit the gathered tensors in SBUF
Host-side prep (layout/quantization only; all model FLOPs stay on device):
  - adjacency degree-normalize + fp8 quantize + transpose; hs fp8 pairs in
    both [h,s] and [s,h] layouts; bf16 triple split of hs for the router
  - expert weights as scaled fp8 pairs; sem/syn pairs row-interleaved to
    match dma_gather's u16-granularity transpose of fp8 data
  - LN gain/bias folded into the syn expert weights
"""

import numpy as np
import ml_dtypes
from contextlib import ExitStack

B, S, H = 8, 1024, 1536
THRESHOLD = 128
P = 128
ST = S // P          # 8 s-tiles
KT = H // P          # 12 h contraction tiles
TT = S // P          # 8 t-tiles for adjacency contraction
NCH = 512            # matmul moving free-dim chunk
NN = H // NCH        # 3 chunks of the H output dim
KD = KT // 2         # 6 DoubleRow passes over H
TD = TT // 2         # 4 DoubleRow passes over S
WS = 32.0            # host-side weight scale for fp8 range
ASC = 256.0          # host-side adjacency scale for fp8 range
EPS = 1e-5
CAP = 512            # per-class token capacity (tokens per region)
RT = CAP // P        # 4 tiles per region
NSLOT = 3 * CAP      # slots per expert group
GT = NSLOT // P      # 12 slot-tiles per group
NW = NSLOT // 16     # idx columns (wrapped in 16 partitions)
CW = NW // 3         # idx columns per region

_BF16 = ml_dtypes.bfloat16
_F8 = ml_dtypes.float8_e4m3

_prog_cache = {}


def _build_program(cfg):
    """cfg = (router_bias_nz, syn_bias_nz, len_bias_nz, sem_bias_nz, cls_bias_nz)"""
    import concourse.bass as bass
    import concourse.tile as tile
    from concourse import bacc, mybir

    rb_nz, synb_nz, lenb_nz, semb_nz, clsb_nz = cfg
    f32 = mybir.dt.float32
    i32 = mybir.dt.int32
    i16 = mybir.dt.int16
    u32 = mybir.dt.uint32
    bf16 = mybir.dt.bfloat16
    fp8 = mybir.dt.float8e4
    AF = mybir.ActivationFunctionType
    ALU = mybir.AluOpType
    AX = mybir.AxisListType
    DR = mybir.MatmulPerfMode.DoubleRow
    ts = bass.ts

    nc = bacc.Bacc("TRN2", target_bir_lowering=False, debug=False,
                   dynamic_dma_scratch_size=24576)

    # ---- DRAM I/O ----
    hsb_d = nc.dram_tensor("hsb", [S, H], bf16, kind="ExternalInput").ap()
    hb1T_d = nc.dram_tensor("hb1T", [H, S], bf16, kind="ExternalInput").ap()
    hb2T_d = nc.dram_tensor("hb2T", [H, S], bf16, kind="ExternalInput").ap()
    hb3T_d = nc.dram_tensor("hb3T", [H, S], bf16, kind="ExternalInput").ap()
    hs8T_d = nc.dram_tensor("hs8T", [H, S], fp8, kind="ExternalInput").ap()
    hsrT_d = nc.dram_tensor("hsrT", [H, S], fp8, kind="ExternalInput").ap()
    hs8r_d = nc.dram_tensor("hs8r", [S, H], fp8, kind="ExternalInput").ap()
    hsrr_d = nc.dram_tensor("hsrr", [S, H], fp8, kind="ExternalInput").ap()
    adjT_d = nc.dram_tensor("adjT", [S, S], fp8, kind="ExternalInput").ap()
    rw1_d = nc.dram_tensor("rw1", [H, 7], bf16, kind="ExternalInput").ap()
    rw2_d = nc.dram_tensor("rw2", [H, 7], bf16, kind="ExternalInput").ap()
    wg1_d = nc.dram_tensor("wg1", [H, H], fp8, kind="ExternalInput").ap()
    wg2_d = nc.dram_tensor("wg2", [H, H], fp8, kind="ExternalInput").ap()
    iotaw_d = nc.dram_tensor("iotaw", [16, S // 16], i32, kind="ExternalInput").ap()
    wexp_d = []  # (w8, wr) per expert: len, sem0-2, syn0-2 (sem/syn interleaved)
    for nm in ["len", "sem0", "sem1", "sem2", "syn0", "syn1", "syn2"]:
        wexp_d.append((
            nc.dram_tensor(f"w{nm}8", [H, H], fp8, kind="ExternalInput").ap(),
            nc.dram_tensor(f"w{nm}r", [H, H], fp8, kind="ExternalInput").ap(),
        ))
    wcls_d = nc.dram_tensor("wcls", [H, 2], bf16, kind="ExternalInput").ap()
    br_d = nc.dram_tensor("br", [1, 7], f32, kind="ExternalInput").ap() if rb_nz else None
    bsyn_d = nc.dram_tensor("bsyn", [3, H], f32, kind="ExternalInput").ap() if synb_nz else None
    blen_d = nc.dram_tensor("blen", [1, H], f32, kind="ExternalInput").ap() if lenb_nz else None
    bsem_d = nc.dram_tensor("bsem", [3, H], f32, kind="ExternalInput").ap() if semb_nz else None
    bcls_d = nc.dram_tensor("bcls", [1, 2], f32, kind="ExternalInput").ap() if clsb_nz else None
    outC_d = nc.dram_tensor("outC", [P, ST, 2], f32, kind="ExternalOutput").ap()
    outB_d = nc.dram_tensor("outB", [P, GT, 2], f32, kind="ExternalOutput").ap()
    outA_d = nc.dram_tensor("outA", [P, GT, 2], f32, kind="ExternalOutput").ap()
    idxsyn_d = nc.dram_tensor("idxsyn", [16, NW], i16, kind="ExternalOutput").ap()
    idxsem_d = nc.dram_tensor("idxsem", [16, NW], i16, kind="ExternalOutput").ap()
    cnts_d = nc.dram_tensor("cnts", [1, 8], u32, kind="ExternalOutput").ap()

    hs_r = hsb_d.rearrange("(a p) h -> p a h", p=P)
    hb_rs = [h.rearrange("(k p) s -> p k s", p=P) for h in (hb1T_d, hb2T_d, hb3T_d)]
    hs8T_r = hs8T_d.rearrange("(k p) s -> p k s", p=P)
    hsrT_r = hsrT_d.rearrange("(k p) s -> p k s", p=P)
    adjT_r = adjT_d.rearrange("(t p) s -> p t s", p=P)
    rw1_r = rw1_d.rearrange("(k p) e -> p k e", p=P)
    rw2_r = rw2_d.rearrange("(k p) e -> p k e", p=P)
    wcls_r = wcls_d.rearrange("(k p) c -> p k c", p=P)
    wg1_r = wg1_d.rearrange("(k p) d -> p k d", p=P)
    wg2_r = wg2_d.rearrange("(k p) d -> p k d", p=P)
    wexp_r = [(w8.rearrange("(k p) d -> p k d", p=P),
               wr.rearrange("(k p) d -> p k d", p=P)) for w8, wr in wexp_d]

    with tile.TileContext(nc) as tc, ExitStack() as ctx:
        # ---- pools ----
        const = ctx.enter_context(tc.tile_pool(name="const", bufs=1))
        hspool = ctx.enter_context(tc.tile_pool(name="hspool", bufs=1))
        f8pool = ctx.enter_context(tc.tile_pool(name="f8pool", bufs=1))
        wpool = ctx.enter_context(tc.tile_pool(name="wpool", bufs=3))
        fpool = ctx.enter_context(tc.tile_pool(name="fpool", bufs=2))
        gpool = ctx.enter_context(tc.tile_pool(name="gpool", bufs=1))
        stage = ctx.enter_context(tc.tile_pool(name="stage", bufs=2))
        small = ctx.enter_context(tc.tile_pool(name="small", bufs=2))
        dram = ctx.enter_context(tc.tile_pool(name="dram", bufs=1, space="DRAM"))
        acc = ctx.enter_context(tc.tile_pool(name="acc", bufs=4, space="PSUM"))
        spsum = ctx.enter_context(tc.tile_pool(name="spsum", bufs=2, space="PSUM"))

        # ---- DRAM scratch (dependency-tracked pool tiles) ----
        shs = dram.tile([S, H], bf16, tag="shs")
        ctab = dram.tile([S, 64], f32, tag="ctab")
        shs_r = shs[:].rearrange("(a p) h -> p a h", p=P)

        # ---- constants (gpsimd DMA queue, parallel to sync queue) ----
        rw1_sb = const.tile([P, KT, 7], bf16, tag="rw1")
        nc.gpsimd.dma_start(rw1_sb[:], rw1_r)
        rw2_sb = const.tile([P, KT, 7], bf16, tag="rw2")
        nc.gpsimd.dma_start(rw2_sb[:], rw2_r)
        wcls_sb = const.tile([P, KT, 2], bf16, tag="wcls")
        nc.gpsimd.dma_start(wcls_sb[:], wcls_r)
        iotaw = const.tile([16, S // 16], i32, tag="iotaw")
        nc.gpsimd.dma_start(iotaw[:], iotaw_d)
        ones_row = None
        if any(x is not None for x in (br_d, bsyn_d, blen_d, bsem_d, bcls_d)):
            ones_row = const.tile([1, P], f32, tag="ones")
            nc.vector.memset(ones_row[:], 1.0)

        def bias_row(dram_ap, n, tag):
            t = const.tile([1, n], f32, tag=tag)
            nc.gpsimd.dma_start(t[:], dram_ap)
            return t

        br_sb = bias_row(br_d, 7, "br") if br_d is not None else None
        blen_sb = bias_row(blen_d, H, "blen") if blen_d is not None else None
        bsem_sb = ([bias_row(bsem_d[e : e + 1, :], H, f"bsem{e}") for e in range(3)]
                   if bsem_d is not None else None)
        bsyn_sb = ([bias_row(bsyn_d[e : e + 1, :], H, f"bsyn{e}") for e in range(3)]
                   if bsyn_d is not None else None)

        # ---- persistent SBUF tensors ----
        hs_all = hspool.tile([P, ST, H], bf16, tag="hs")      # hs -> resid -> fusedC
        hs8T = f8pool.tile([P, KT, S], fp8, tag="hs8T")
        hsrT = f8pool.tile([P, KT, S], fp8, tag="hsrT")
        adjT = f8pool.tile([P, TT, S], fp8, tag="adjT")       # 256 * Anorm^T

        # ---- weight chunk streaming ([P, KT, NCH] slices, consumption order) --
        # all wpool tiles (weight chunks + router hb slices) alternate between
        # the two rotating tag slots in issue order == consumption order
        _tag_ctr = [0]

        def next_tag():
            _tag_ctr[0] += 1
            return "w8" if _tag_ctr[0] % 2 else "wr"

        def load_wc(wdram_r, n, tag=None):
            wt = wpool.tile([P, KT, NCH], fp8, tag=next_tag(), name=f"wc{_tag_ctr[0]}")
            nc.sync.dma_start(wt[:], wdram_r[:, :, ts(n, NCH)])
            return wt

        # router hb k-slices stream through a small pool; DMAs are interleaved
        # with the GCN chunk loads and consumed by router k-blocks interleaved
        # with the GCN matmul phases (so neither queue stalls the other)
        hb_sl = [[None] * KT for _ in range(3)]

        def load_hb(k):
            for t in range(3):
                sl = wpool.tile([P, S], bf16, tag=next_tag(), name=f"hb{t}_{k}")
                nc.sync.dma_start(sl[:], hb_rs[t][:, k, :])
                hb_sl[t][k] = sl

        # GCN1 chunk 0 + full hs8T first (needed for the first psum group)
        nc.sync.dma_start(hs8T[:, 0:6, :], hs8T_r[:, 0:6, :])
        wg1_c0 = load_wc(wg1_r, 0)
        nc.sync.dma_start(hs8T[:, 6:12, :], hs8T_r[:, 6:12, :])
        wg1_c = [wg1_c0, load_wc(wg1_r, 1)]
        wg1_c.append(load_wc(wg1_r, 2))
        nc.sync.dma_start(adjT[:], adjT_r)
        wg2_c = [load_wc(wg2_r, n) for n in range(NN)]
        for m in range(3):
            nc.sync.dma_start(hs_all[:, m, :], hs_r[:, m, :])
        load_hb(0)
        load_hb(1)
        load_hb(2)
        load_hb(3)
        nc.sync.dma_start(hs_all[:, 3, :], hs_r[:, 3, :])
        load_hb(4)
        load_hb(5)
        nc.sync.dma_start(hs_all[:, 4, :], hs_r[:, 4, :])
        load_hb(6)
        load_hb(7)
        nc.sync.dma_start(hs_all[:, 5, :], hs_r[:, 5, :])
        load_hb(8)
        load_hb(9)
        nc.sync.dma_start(hs_all[:, 6, :], hs_r[:, 6, :])
        load_hb(10)
        load_hb(11)
        nc.sync.dma_start(hs_all[:, 7, :], hs_r[:, 7, :])
        nc.sync.dma_start(hsrT[:], hsrT_r)
        # expert weight chunks: len, sem0-2, syn0-2; (w8, wr) pairs per n-chunk
        wexp_c = []
        for ei in range(7):
            cs = []
            for n in range(NN):
                cs.append((load_wc(wexp_r[ei][0], n),
                           load_wc(wexp_r[ei][1], n)))
            wexp_c.append(cs)

        # ---- router k-blocks (fp32-exact logits from bf16 triple/pair split) --
        logit = small.tile([P, ST, 7], f32, tag="logit", bufs=1)
        nc.vector.memset(logit[:], 0.0)

        def router_k(k):
            rlog = spsum.tile([P, ST, 7], f32, tag="sp")
            terms = ((hb_sl[0][k], rw1_sb), (hb_sl[0][k], rw2_sb),
                     (hb_sl[1][k], rw1_sb), (hb_sl[1][k], rw2_sb),
                     (hb_sl[2][k], rw1_sb))
            for m in range(ST):
                for t_i, (hb, rwt) in enumerate(terms):
                    nc.tensor.matmul(rlog[:, m, :], hb[:, ts(m, P)],
                                     rwt[:, k, :], start=(t_i == 0),
                                     stop=(t_i == len(terms) - 1))
            nc.vector.tensor_add(logit[:], logit[:], rlog[:])

        # ---- GCN: S1 = hs8 @ W1q (fp8 DR), evict /32 -> fp8 [s, d] ----
        s_sb = f8pool.tile([P, ST, H], fp8, tag="s12")
        for n in range(NN):
            for m in range(ST):
                ps = acc.tile([P, NCH], f32, tag="acc")
                for j in range(KD):
                    nc.tensor.matmul(ps[:], hs8T[:, 2 * j : 2 * j + 2, ts(m, P)],
                                     wg1_c[n][:, 2 * j : 2 * j + 2, :],
                                     start=(j == 0), stop=(j == KD - 1), perf_mode=DR)
                nc.scalar.activation(s_sb[:, m, ts(n, NCH)], ps[:], AF.Copy,
                                     scale=1.0 / WS)

        # ---- x1T = relu(Anorm @ S1)^T via lhsT=S1: store 32*relu(x1) fp8 ----
        x1T = f8pool.tile([P, KT, S], fp8, tag="x1T")
        for dt_i in range(KT):
            for sc in range(2):
                ps = acc.tile([P, NCH], f32, tag="acc")
                for j in range(TD):
                    nc.tensor.matmul(ps[:], s_sb[:, 2 * j : 2 * j + 2, ts(dt_i, P)],
                                     adjT[:, 2 * j : 2 * j + 2, ts(sc, NCH)],
                                     start=(j == 0), stop=(j == TD - 1), perf_mode=DR)
                nc.scalar.activation(x1T[:, dt_i, ts(sc, NCH)], ps[:], AF.Relu,
                                     scale=WS / ASC)

        # ---- S2 = (32 x1) @ W2q: store 32*S2 in fp8 ----
        s2_sb = f8pool.tile([P, ST, H], fp8, tag="s12")
        for n in range(NN):
            for m in range(ST):
                ps = acc.tile([P, NCH], f32, tag="acc")
                for j in range(KD):
                    nc.tensor.matmul(ps[:], x1T[:, 2 * j : 2 * j + 2, ts(m, P)],
                                     wg2_c[n][:, 2 * j : 2 * j + 2, :],
                                     start=(j == 0), stop=(j == KD - 1), perf_mode=DR)
                nc.scalar.activation(s2_sb[:, m, ts(n, NCH)], ps[:], AF.Copy,
                                     scale=1.0 / WS)

        # ---- residual + LayerNorm -> sh (bf16) ----
        sh_t = [None] * ST

        def do_ln(m):
            stats = small.tile([P, NN, 6], bf16, tag="stats")
            for c in range(NN):
                nc.vector.bn_stats(stats[:, c, :], hs_all[:, m, ts(c, NCH)])
            mv = small.tile([P, 2], f32, tag="mv")
            nc.vector.bn_aggr(mv[:], stats[:])
            # rsqrt via bit-trick seed + Newton step on DVE (keeps Sqrt off Act)
            veps = small.tile([P, 1], f32, tag="veps")
            nc.vector.tensor_scalar(out=veps[:], in0=mv[:, 1:2], scalar1=EPS,
                                    scalar2=None, op0=ALU.add)
            rsd_i = small.tile([P, 1], i32, tag="rsdi")
            nc.vector.tensor_scalar(out=rsd_i[:], in0=veps[:].bitcast(i32),
                                    scalar1=1, scalar2=None,
                                    op0=ALU.logical_shift_right)
            nc.vector.tensor_scalar(out=rsd_i[:], in0=rsd_i[:], scalar1=-1,
                                    scalar2=0x5F3759DF, op0=ALU.mult, op1=ALU.add)
            rstd = rsd_i[:].bitcast(f32)
            nwt = small.tile([P, 1], f32, tag="nwt")
            nc.vector.tensor_mul(nwt[:], rstd, rstd)
            nc.vector.tensor_mul(nwt[:], nwt[:], veps[:])
            nc.vector.tensor_scalar(out=nwt[:], in0=nwt[:], scalar1=-0.5,
                                    scalar2=1.5, op0=ALU.mult, op1=ALU.add)
            nc.vector.tensor_mul(rstd, rstd, nwt[:])
            nmr = small.tile([P, 1], f32, tag="nmr")
            nc.vector.tensor_mul(nmr[:], mv[:, 0:1], rstd)
            nc.vector.tensor_scalar_mul(nmr[:], nmr[:], -1.0)
            sh = stage.tile([P, H], bf16, tag="shm", bufs=2)
            nc.scalar.activation(sh[:], hs_all[:, m, :], AF.Identity,
                                 bias=nmr[:], scale=rstd)
            nc.gpsimd.dma_start(shs_r[:, m, :], sh[:])
            sh_t[m] = sh

        # ---- x2: resid += relu(psum)/8192 (residual adds on DVE) ----
        for m in range(ST):
            for n in range(NN):
                ps = acc.tile([P, NCH], f32, tag="acc")
                for j in range(TD):
                    nc.tensor.matmul(ps[:], adjT[:, 2 * j : 2 * j + 2, ts(m, P)],
                                     s2_sb[:, 2 * j : 2 * j + 2, ts(n, NCH)],
                                     start=(j == 0), stop=(j == TD - 1), perf_mode=DR)
                g = stage.tile([P, NCH], f32, tag="hTf", bufs=3)
                nc.scalar.activation(g[:], ps[:], AF.Relu, scale=1.0 / (ASC * WS))
                nc.vector.tensor_add(hs_all[:, m, ts(n, NCH)],
                                     hs_all[:, m, ts(n, NCH)], g[:])
                gidx = m * NN + n
                if gidx % 2 == 0 and gidx // 2 < KT:
                    router_k(gidx // 2)
                if n == NN - 1:
                    do_ln(m)

        if br_sb is not None:
            rlog = spsum.tile([P, ST, 7], f32, tag="sp")
            for m in range(ST):
                nc.tensor.matmul(rlog[:, m, :], ones_row[:], br_sb[:],
                                 start=True, stop=True)
            nc.vector.tensor_add(logit[:], logit[:], rlog[:])

        # ---- router math: group softmax ratios + top-1 coefficients ----
        # coef table layout (64 f32 per token): 0=clen, 1..3=csyn, 4..6=csem
        coef_sb = small.tile([P, ST, 64], f32, tag="ctabs", bufs=1)
        nc.vector.memset(coef_sb[:], 0.0)
        e_sb = small.tile([P, ST, 7], f32, tag="esb")
        nc.scalar.activation(e_sb[:], logit[:], AF.Exp)
        gdum = small.tile([1, 4], f32, tag="gdum", bufs=1)
        nc.scalar.activation(gdum[:], e_sb[0:1, 0, 0:4], AF.Gelu)
        syn_e = small.tile([P, ST], f32, tag="syn_e")
        nc.vector.tensor_reduce(syn_e[:], e_sb[:, :, 0:3], axis=AX.X, op=ALU.max)
        sem_e = small.tile([P, ST], f32, tag="sem_e")
        nc.vector.tensor_reduce(sem_e[:], e_sb[:, :, 4:7], axis=AX.X, op=ALU.max)
        rden = small.tile([P, ST], f32, tag="rden")
        nc.vector.tensor_add(rden[:], syn_e[:], sem_e[:])
        nc.vector.tensor_add(rden[:], rden[:], e_sb[:, :, 3])
        nc.vector.reciprocal(rden[:], rden[:])
        nc.vector.tensor_mul(coef_sb[:, :, 0], e_sb[:, :, 3], rden[:])

        cls_f = [None, None]  # f32 class vecs: [syn, sem]

        def group_coefs(gi, ccol, base, w_e):
            """coef cols ccol..ccol+2 = rden * w_e * mask_e; class vec = first-max
            argmax over logit columns base..base+2 (matches jnp tie-breaking)."""
            l0, l1, l2 = (logit[:, :, base + i] for i in range(3))
            s0 = small.tile([P, ST], f32, tag="s0")
            ge02 = small.tile([P, ST], f32, tag="ge02")
            nc.vector.tensor_tensor(out=s0[:], in0=l0, in1=l1, op=ALU.is_ge)
            nc.vector.tensor_tensor(out=ge02[:], in0=l0, in1=l2, op=ALU.is_ge)
            nc.vector.tensor_mul(s0[:], s0[:], ge02[:])
            s1 = small.tile([P, ST], f32, tag="s1")
            ge12 = small.tile([P, ST], f32, tag="ge12")
            nc.vector.tensor_tensor(out=ge12[:], in0=l1, in1=l2, op=ALU.is_ge)
            nc.vector.tensor_mul(s1[:], s0[:], ge12[:])
            nc.vector.tensor_tensor(out=s1[:], in0=ge12[:], in1=s1[:], op=ALU.subtract)
            s2 = small.tile([P, ST], f32, tag="s2")
            nc.vector.tensor_add(s2[:], s0[:], s1[:])
            nc.vector.tensor_scalar(out=s2[:], in0=s2[:], scalar1=-1.0, scalar2=1.0,
                                    op0=ALU.mult, op1=ALU.add)
            for e, sm in enumerate((s0, s1, s2)):
                nc.vector.tensor_mul(coef_sb[:, :, ccol + e], sm[:], w_e)
                nc.vector.tensor_mul(coef_sb[:, :, ccol + e],
                                     coef_sb[:, :, ccol + e], rden[:])
            cg = small.tile([P, ST], f32, tag=f"clsv{gi}", bufs=1)
            nc.vector.tensor_scalar(out=cg[:], in0=s2[:], scalar1=2.0, scalar2=None,
                                    op0=ALU.mult)
            nc.vector.tensor_add(cg[:], cg[:], s1[:])
            cls_f[gi] = cg

        group_coefs(0, 1, 0, syn_e[:])
        group_coefs(1, 4, 4, sem_e[:])
        nc.gpsimd.dma_start(ctab[:].rearrange("(a p) c -> p a c", p=P), coef_sb[:])

        # ---- counting sort per group: wrapped class -> sparse_gather lists ----
        cnts_sb = small.tile([1, 8], u32, tag="cnts", bufs=1)
        nc.vector.memset(cnts_sb[:], 0)
        idx_tiles = []
        for gi in range(2):  # 0=syn, 1=sem
            cg_i = small.tile([P, ST], i32, tag=f"cgi{gi}", bufs=1)
            nc.vector.tensor_copy(cg_i[:], cls_f[gi][:])
            clsw = small.tile([16, ST, 8], i32, tag=f"clsw{gi}", bufs=1)
            for r in range(8):
                nc.gpsimd.dma_start(clsw[:, :, r], cg_i[r * 16:(r + 1) * 16, :])
            clsw_f = clsw[:].rearrange("q a r -> q (a r)")
            arr = small.tile([16, 3, S // 16], f32, tag=f"arr{gi}", bufs=1)
            msk = small.tile([16, S // 16], i32, tag="msk")
            iop = small.tile([16, S // 16], i32, tag="iop")
            for c in range(3):
                nc.vector.tensor_scalar(out=msk[:], in0=clsw_f, scalar1=c,
                                        scalar2=None, op0=ALU.is_equal)
                nc.vector.tensor_scalar(out=iop[:], in0=iotaw[:], scalar1=1,
                                        scalar2=None, op0=ALU.add)
                nc.vector.tensor_tensor(out=iop[:], in0=msk[:], in1=iop[:],
                                        op=ALU.mult)
                nc.vector.tensor_scalar(out=arr[:, c, :], in0=iop[:], scalar1=-1,
                                        scalar2=None, op0=ALU.add)
            glist = small.tile([16, 3, S // 16], f32, tag=f"gl{gi}", bufs=1)
            for c in range(3):
                nc.gpsimd.sparse_gather(
                    glist[:, c, :], arr[:, c, :],
                    num_found=cnts_sb[0:1, 3 * gi + c : 3 * gi + c + 1])
            gmax = small.tile([16, 3, CW], f32, tag="gmax")
            nc.vector.tensor_scalar(out=gmax[:], in0=glist[:, :, 0:CW],
                                    scalar1=0.0, scalar2=float(S - 1),
                                    op0=ALU.max, op1=ALU.min)
            idxs = gpool.tile([P, NW], i16, tag=f"idx{gi}", name=f"idx{gi}")
            nc.vector.tensor_copy(idxs[0:16, :], gmax[:].rearrange("q c n -> q (c n)"))
            for g in range(1, 8):
                nc.gpsimd.dma_start(idxs[g * 16:(g + 1) * 16, :], idxs[0:16, :])
            idx_tiles.append(idxs)
            nc.sync.dma_start(idxsyn_d if gi == 0 else idxsem_d, idxs[0:16, :])
        nc.sync.dma_start(cnts_d, cnts_sb[:])
        idx_syn, idx_sem = idx_tiles

        # ---- indirect gathers: region x tensors + coef tables ----
        # gathered layout per region: [p, f, s, b] with h = 256f + 2p + b
        def gather_x(src_dram, idxs, tags, pool):
            tiles = []
            for c in range(3):
                t = pool.tile([P, 6, CAP, 2], fp8, tag=tags[c], name=f"{tags[c]}x")
                gv = t[:].rearrange("p f s b -> p (f s b)").rearrange(
                    "p (t n) -> p t n", t=KT)
                nc.gpsimd.dma_gather(gv, src_dram, idxs[:, c * CW:(c + 1) * CW],
                                     CAP, CAP, H, elem_step=H, transpose=True)
                tiles.append(t)
            return tiles

        def gather_coef(idxs, tag):
            t = gpool.tile([P, GT, 64], f32, tag=tag, name=tag)
            for c in range(3):
                nc.gpsimd.dma_gather(t[:, c * RT:(c + 1) * RT, :], ctab[:],
                                     idxs[:, c * CW:(c + 1) * CW],
                                     CAP, CAP, 64, elem_step=64, transpose=False)
            return t

        xg8_sem = gather_x(hs8r_d, idx_sem, ["s12", "x1T", "adjT"], f8pool)
        xgr_sem = gather_x(hsrr_d, idx_sem, ["xgb0", "xgb1", "xgb2"], gpool)
        coefp_sem = gather_coef(idx_sem, "cpsem")
        coefp_syn = gather_coef(idx_syn, "cpsyn")


        # ---- len expert (dense, original token order) into hs_all ----

        x8, xr = hs8T, hsrT
        for n in range(NN):
            w8c, wrc = wexp_c[0][n]
            for m in range(ST):
                ps = acc.tile([P, NCH], f32, tag="acc")
                for t_i, (xx, ww) in enumerate(((x8, w8c), (xr, w8c), (x8, wrc))):
                    for j in range(KD):
                        last = (t_i == 2 and j == KD - 1 and blen_sb is None)
                        nc.tensor.matmul(
                            ps[:], xx[:, 2 * j : 2 * j + 2, ts(m, P)],
                            ww[:, 2 * j : 2 * j + 2, :],
                            start=(t_i == 0 and j == 0), stop=last, perf_mode=DR)
                if blen_sb is not None:
                    nc.tensor.matmul(ps[:], ones_row[:], blen_sb[:, ts(n, NCH)],
                                     start=False, stop=True)
                g = stage.tile([P, NCH], f32, tag="hTf", bufs=3)
                nc.scalar.activation(g[:], ps[:], AF.Gelu, scale=1.0 / WS)
                nc.vector.tensor_scalar_mul(hs_all[:, m, ts(n, NCH)], g[:],
                                            coef_sb[:, m, 0:1])

        # ---- cls tail helpers (linear head applied per partial output) ----
        out_sbC = small.tile([P, ST, 2], f32, tag="outC", bufs=1)
        out_sbB = small.tile([P, GT, 2], f32, tag="outB", bufs=1)
        out_sbA = small.tile([P, GT, 2], f32, tag="outA", bufs=1)
        bcls_sb = bias_row(bcls_d, 2, "bcls") if bcls_d is not None else None

        def make_tail(out_sb, with_bias):
            fuT_t = {}

            def pre(src_ap, m):
                fuT = stage.tile([P, KT, P], bf16, tag="fuT", bufs=4)
                nc.scalar.dma_start_transpose(fuT[:], src_ap)
                fuT_t[m] = fuT

            def cls(m):
                fuT = fuT_t.pop(m)
                cps = spsum.tile([P, 2], f32, tag="cls")
                for k in range(KT):
                    last = (k == KT - 1) and not (with_bias and bcls_sb is not None)
                    nc.tensor.matmul(cps[:], fuT[:, k, :], wcls_sb[:, k, :],
                                     start=(k == 0), stop=last)
                if with_bias and bcls_sb is not None:
                    nc.tensor.matmul(cps[:], ones_row[:], bcls_sb[:],
                                     start=False, stop=True)
                nc.vector.tensor_copy(out_sb[:, m, :], cps[:])

            return pre, cls

        preC, clsC = make_tail(out_sbC, True)
        preB, clsB = make_tail(out_sbB, False)
        preA, clsA = make_tail(out_sbA, False)

        # ---- region experts: region c of a group evaluates expert c ----
        # (per-slot coefs are zero for other-class tokens / dead pad slots)
        def region_expert(xg8, xgr, wcs, coefp, ccol, c, bias_sb, after_tile,
                          il=True):
            fb = fpool.tile([P, RT, H], bf16, tag="fus", name=f"fus{ccol}_{c}")
            for n in range(NN):
                w8c, wrc = wcs[n]
                for mm in range(RT):
                    ps = acc.tile([P, NCH], f32, tag="acc")
                    k = 0
                    for xx, ww in ((xg8, w8c), (xgr, w8c), (xg8, wrc)):
                        for jj in range(KD):
                            k += 1
                            last = (k == 18 and bias_sb is None)
                            if il:
                                bb, f0 = jj // 3, 2 * (jj % 3)
                                lhsT = xx[:, f0:f0 + 2, ts(mm, P), bb]
                                rhs = ww[:, 6 * bb + f0 : 6 * bb + f0 + 2, :]
                            else:
                                lhsT = xx[:, 2 * jj:2 * jj + 2, ts(mm, P)]
                                rhs = ww[:, 2 * jj:2 * jj + 2, :]
                            nc.tensor.matmul(ps[:], lhsT, rhs,
                                             start=(k == 1), stop=last,
                                             perf_mode=DR)
                    if bias_sb is not None:
                        nc.tensor.matmul(ps[:], ones_row[:], bias_sb[:, ts(n, NCH)],
                                         start=False, stop=True)
                    g = stage.tile([P, NCH], f32, tag="hTf", bufs=3)
                    nc.scalar.activation(g[:], ps[:], AF.Gelu, scale=1.0 / WS)
                    nc.vector.tensor_scalar_mul(
                        fb[:, mm, ts(n, NCH)], g[:],
                        coefp[:, c * RT + mm, ccol:ccol + 1])
                    after_tile(fb, n * RT + mm, c * RT + mm)

        # sem phase: fusedB region tails + fusedC (hs_all) tails, with cls
        # trailing its transpose by two hooks to hide the fuT DMA latency
        def sem_after(fb, si, gm):
            if si < 2 * RT:
                return
            mm = gm % RT
            preB(fb[:, mm, :], gm)
            if gm >= 2:
                clsB(gm - 2)
            if gm < ST:
                preC(hs_all[:, gm, :], gm)
            if gm >= 3 and gm - 3 < ST:
                clsC(gm - 3)
            if gm == ST + 2:
                nc.sync.dma_start(outC_d, out_sbC[:])

        # syn-group gather+split: bf16 transpose-gather of shared rows into a
        # temp (standard k-tile layout), then split to the fp8 pair on-chip;
        # issued between sem regions so each region's WAR deps resolve in turn
        xg8_syn = [None] * 3
        xgr_syn = [None] * 3
        syn_tmp = [None] * 3

        def syn_gather(c):
            tmp = f8pool.tile([P, KT, CAP], bf16, tag="hs8T" if c % 2 == 0 else "hsrT",
                              name=f"sgt{c}")
            nc.gpsimd.dma_gather(tmp[:], shs[:], idx_syn[:, c * CW:(c + 1) * CW],
                                 CAP, CAP, H, elem_step=H, transpose=True)
            syn_tmp[c] = tmp
            x8 = f8pool.tile([P, KT, CAP], fp8, tag=("s12", "x1T", "adjT")[c],
                             name=f"sg8{c}")
            xr = gpool.tile([P, KT, CAP], fp8, tag=f"xgb{c}", name=f"sgr{c}")
            xg8_syn[c] = x8
            xgr_syn[c] = xr

        def split_views(c, h):
            tmp3 = syn_tmp[c][:].rearrange("p k (q n) -> p (k q) n", q=RT)
            x83 = xg8_syn[c][:].rearrange("p k (q n) -> p (k q) n", q=RT)
            xr3 = xgr_syn[c][:].rearrange("p k (q n) -> p (k q) n", q=RT)
            part = slice(h * 4, h * 4 + 4)
            return tmp3[:, part, :], x83[:, part, :], xr3[:, part, :]

        def split_copy(c, h):
            tmp_p, x8_p, _ = split_views(c, h)
            nc.scalar.activation(x8_p, tmp_p, AF.Copy)

        def split_sub(c, h):
            tmp_p, x8_p, xr_p = split_views(c, h)
            nc.vector.tensor_tensor(out=xr_p, in0=tmp_p, in1=x8_p,
                                    op=ALU.subtract)

        # sem r0: plain; sem r1 hooks split c0; sem r2 hooks split c1
        split_sched = {1: 0, 2: 1}

        def make_sem_after(creg):
            def hook(fb, gm):
                sem_after(fb, gm)
                if creg in split_sched:
                    split_piece(split_sched[creg], gm % GT if False else (gm - creg * RT) + ((gm // RT) - creg) * 0 + 0)
            return hook

        step_ctr = [0]

        def sem_hook_factory(creg):
            def hook(fb, gm):
                sem_after(fb, gm)
            return hook

        for c in range(3):
            # interleave split pieces of region c-1's syn input into this
            # region's eval steps (slots freed at region c-1's end)
            cc = c - 1  # split pieces for the previous region's syn input

            def hook(fb, si, gm, cc=cc):
                sem_after(fb, si, gm)
                if cc < 0:
                    return
                split_copy(cc, si)
                if si >= 2:
                    split_sub(cc, si - 2)
                if si == GT - 1:
                    split_sub(cc, GT - 2)
                    split_sub(cc, GT - 1)

            region_expert(xg8_sem[c], xgr_sem[c], wexp_c[1 + c], coefp_sem,
                          4 + c, c, bsem_sb[c] if bsem_sb else None, hook)
            syn_gather(c)

        def syn_after(fa, si, gm):
            if si < 2 * RT:
                return
            mm = gm % RT
            preA(fa[:, mm, :], gm)
            if gm == 0:
                clsB(GT - 2)
            elif gm == 1:
                clsB(GT - 1)
                nc.sync.dma_start(outB_d, out_sbB[:])
            if gm >= 2:
                clsA(gm - 2)

        for c in range(3):
            def hook(fa, si, gm, c=c):
                syn_after(fa, si, gm)
                if c != 0:
                    return
                split_copy(2, si)
                if si >= 2:
                    split_sub(2, si - 2)
                if si == GT - 1:
                    split_sub(2, GT - 2)
                    split_sub(2, GT - 1)

            region_expert(xg8_syn[c], xgr_syn[c], wexp_c[4 + c], coefp_syn,
                          1 + c, c, bsyn_sb[c] if bsyn_sb else None, hook,
                          il=False)
        clsA(GT - 2)
        clsA(GT - 1)
        nc.sync.dma_start(outA_d, out_sbA[:])

    nc.compile()
    return nc


def _get_program(cfg):
    if cfg not in _prog_cache:
        _prog_cache[cfg] = _build_program(cfg)
    return _prog_cache[cfg]


def _fp8_pair(w):
    """w -> (q8(32w), q8(32w - float(q8(32w)))) as contiguous fp8 arrays."""
    ws = (WS * w).astype(np.float32)
    w8 = ws.astype(_F8)
    wr = (ws - w8.astype(np.float32)).astype(_F8)
    return np.ascontiguousarray(w8), np.ascontiguousarray(wr)


# dma_gather transposes fp8 at u16 granularity: gathered[p, f, s, b] holds
# h = 256f + 2p + b; sem/syn weight rows are pre-permuted to match, with
# k-tile t = 6b + f containing rows 256f + 2p + b.
_IL_ROWS = np.empty(H, dtype=np.int64)
for _b in range(2):
    for _f in range(6):
        _IL_ROWS[(6 * _b + _f) * P:(6 * _b + _f + 1) * P] = \
            256 * _f + 2 * np.arange(P) + _b


def _fp8_pair_il(w):
    w8, wr = _fp8_pair(w)
    return (np.ascontiguousarray(w8[_IL_ROWS, :]),
            np.ascontiguousarray(wr[_IL_ROWS, :]))


def kernel(**inputs):
    from concourse import bass_utils

    hs = np.asarray(inputs["hidden_states"], dtype=np.float32)
    adj = np.asarray(inputs["adj_matrix"], dtype=np.float32)
    seq_lengths = np.asarray(inputs["seq_lengths"])
    router_w = np.asarray(inputs["router_w"], dtype=np.float32)
    router_b = np.asarray(inputs["router_b"], dtype=np.float32)
    gcn1_w = np.asarray(inputs["gcn1_w"], dtype=np.float32)
    gcn2_w = np.asarray(inputs["gcn2_w"], dtype=np.float32)
    ln_g = np.asarray(inputs["ln_g"], dtype=np.float32)
    ln_b = np.asarray(inputs["ln_b"], dtype=np.float32)
    syn_w = np.asarray(inputs["syn_w"], dtype=np.float32)
    syn_b = np.asarray(inputs["syn_b"], dtype=np.float32)
    len_short_w = np.asarray(inputs["len_short_w"], dtype=np.float32)
    len_short_b = np.asarray(inputs["len_short_b"], dtype=np.float32)
    len_long_w = np.asarray(inputs["len_long_w"], dtype=np.float32)
    len_long_b = np.asarray(inputs["len_long_b"], dtype=np.float32)
    sem_w = np.asarray(inputs["sem_w"], dtype=np.float32)
    sem_b = np.asarray(inputs["sem_b"], dtype=np.float32)
    cls_w = np.asarray(inputs["cls_w"], dtype=np.float32)
    cls_b = np.asarray(inputs["cls_b"], dtype=np.float32)

    # fold LN affine into syn expert weights
    syn_w_f = (ln_g[None, :, None] * syn_w).astype(np.float32)
    syn_b_f = (syn_b + np.einsum("h,ehd->ed", ln_b, syn_w)).astype(np.float32)

    is_short = seq_lengths <= THRESHOLD

    cfg = (
        bool(np.any(router_b != 0)),
        bool(np.any(syn_b_f != 0)),
        bool(np.any(len_short_b != 0) or np.any(len_long_b != 0)),
        bool(np.any(sem_b != 0)),
        bool(np.any(cls_b != 0)),
    )
    nc = _get_program(cfg)

    wg1_8, _ = _fp8_pair(gcn1_w)
    wg2_8, _ = _fp8_pair(gcn2_w)
    wls = _fp8_pair(len_short_w)
    wll = _fp8_pair(len_long_w)
    wsem = [_fp8_pair_il(sem_w[e]) for e in range(3)]
    wsyn = [_fp8_pair(syn_w_f[e]) for e in range(3)]
    wcls = np.ascontiguousarray(cls_w.astype(_BF16))
    iotaw = np.ascontiguousarray(
        np.arange(S, dtype=np.int32).reshape(S // 16, 16).T)

    in_maps = []
    for b in range(B):
        lencol = 3 if is_short[b] else 4
        rw7 = np.ascontiguousarray(np.concatenate(
            [router_w[:, 0:3], router_w[:, lencol : lencol + 1], router_w[:, 5:8]],
            axis=1, dtype=np.float32))
        wlen = wls if is_short[b] else wll
        hsb = hs[b]
        hs8 = hsb.astype(_F8)
        hsr = (hsb - hs8.astype(np.float32)).astype(_F8)
        hb1 = hsb.astype(_BF16)
        r = hsb - hb1.astype(np.float32)
        hb2 = r.astype(_BF16)
        hb3 = (r - hb2.astype(np.float32)).astype(_BF16)
        rw1 = rw7.astype(_BF16)
        rw2 = (rw7 - rw1.astype(np.float32)).astype(_BF16)
        deg = np.clip(adj[b].sum(axis=1, keepdims=True), 1e-9, None)
        adjq = (ASC * adj[b] / deg).astype(_F8)
        m = {
            "hsb": np.ascontiguousarray(hb1),
            "hb1T": np.ascontiguousarray(hb1.T),
            "hb2T": np.ascontiguousarray(hb2.T),
            "hb3T": np.ascontiguousarray(hb3.T),
            "hs8T": np.ascontiguousarray(hs8.T),
            "hsrT": np.ascontiguousarray(hsr.T),
            "hs8r": np.ascontiguousarray(hs8),
            "hsrr": np.ascontiguousarray(hsr),
            "adjT": np.ascontiguousarray(adjq.T),
            "rw1": np.ascontiguousarray(rw1),
            "rw2": np.ascontiguousarray(rw2),
            "wg1": wg1_8, "wg2": wg2_8,
            "wlen8": wlen[0], "wlenr": wlen[1],
            "wcls": wcls,
            "iotaw": iotaw,
        }
        for e in range(3):
            m[f"wsem{e}8"], m[f"wsem{e}r"] = wsem[e]
            m[f"wsyn{e}8"], m[f"wsyn{e}r"] = wsyn[e]
        if cfg[0]:
            br7 = np.concatenate(
                [router_b[0:3], router_b[lencol : lencol + 1], router_b[5:8]])
            m["br"] = br7.reshape(1, 7).astype(np.float32)
        if cfg[1]:
            m["bsyn"] = (WS * syn_b_f).astype(np.float32)
        if cfg[2]:
            m["blen"] = (WS * (len_short_b if is_short[b]
                               else len_long_b)).reshape(1, H).astype(np.float32)
        if cfg[3]:
            m["bsem"] = (WS * sem_b).astype(np.float32)
        if cfg[4]:
            m["bcls"] = cls_b.reshape(1, 2).astype(np.float32)
        in_maps.append(m)

    try:
        res = bass_utils.run_bass_kernel_spmd(nc, in_maps, core_ids=list(range(B)))
    except Exception:
        # transient device wedge (NRT_EXEC_UNIT_UNRECOVERABLE) clears on retry
        res = bass_utils.run_bass_kernel_spmd(nc, in_maps, core_ids=list(range(B)))
    globals()["_last_results"] = res

    out = np.empty((B, S, 2), dtype=np.float32)
    for b in range(B):
        r = res.results[b]
        outC = np.asarray(r["outC"], dtype=np.float32).transpose(1, 0, 2).reshape(S, 2)
        outB = np.asarray(r["outB"], dtype=np.float32).transpose(1, 0, 2).reshape(NSLOT, 2)
        outA = np.asarray(r["outA"], dtype=np.float32).transpose(1, 0, 2).reshape(NSLOT, 2)
        cnts = np.asarray(r["cnts"]).ravel()
        acc = outC.copy()
        for gi, outX, idx_name in ((0, outA, "idxsyn"), (1, outB, "idxsem")):
            idxw = np.asarray(r[idx_name])
            idx_un = idxw.T.reshape(-1).astype(np.int64)  # slot i = f*16 + q
            for c in range(3):
                nlive = min(int(cnts[3 * gi + c]), CAP)
                slots = np.arange(c * CAP, c * CAP + nlive)
                acc[idx_un[slots]] += outX[slots]
        out[b] = acc
    return out


# revision 33
# speedup vs baseline: 1.0526x; 1.0178x over previous
"""Trainium2 Bass kernel for nn_MoEDetector (moe_routing).

Strategy: data-parallel over batch B=8 -> one batch per NeuronCore, with
top-1 sparse expert dispatch on-device:
  - router logits in fp32 (argmax-selection safe), group softmax ratios
  - GCN chain in single-term fp8 (output is ~5e-4 of the residual stream)
  - tokens are counting-sorted by their syn/sem argmax class on device
    (sparse_gather) into 3 fixed 512-token capacity regions per group;
    dma_gather (indirect DMA) fetches each region's tokens from DRAM in
    transposed [h, slot] layout at zero PE cost
  - each region statically maps to one expert, so the region matmuls run
    at full fp8 DoubleRow speed with 3-term splits (X@W ~ X8@W8 + Xr@W8 +
    X8@Wr); 24 region-tile evals replace 48 dense sem+syn evals
  - the len expert choice is forced per-batch by seq_lengths via router
    masking, so it runs dense in original token order
  - cls head is linear: three partial outputs (len / sem / syn order) are
    produced separately and summed on host after unpermuting
  - expert weights and router hb-splits stream through SBUF in 512-column
    chunks (n-outer loops) to fit the gathered tensors in SBUF
Host-side prep (layout/quantization only; all model FLOPs stay on device):
  - adjacency degree-normalize + fp8 quantize + transpose; hs fp8 pairs in
    both [h,s] and [s,h] layouts; bf16 triple split of hs for the router
  - expert weights as scaled fp8 pairs; sem/syn pairs row-interleaved to
    match dma_gather's u16-granularity transpose of fp8 data
  - LN gain/bias folded into the syn expert weights
"""

import numpy as np
import ml_dtypes
from contextlib import ExitStack

B, S, H = 8, 1024, 1536
THRESHOLD = 128
P = 128
ST = S // P          # 8 s-tiles
KT = H // P          # 12 h contraction tiles
TT = S // P          # 8 t-tiles for adjacency contraction
NCH = 512            # adjacency-matmul moving free-dim chunk
NN = H // NCH        # 3 chunks of the H output dim
WCH = 512            # weight-streamed matmul chunk (S1/S2/len/regions)
WNN = H // WCH       # 3 chunks
KD = KT // 2         # 6 DoubleRow passes over H
TD = TT // 2         # 4 DoubleRow passes over S
WS = 32.0            # host-side weight scale for fp8 range
ASC = 256.0          # host-side adjacency scale for fp8 range
EPS = 1e-5
CAP = 512            # per-class token capacity (tokens per region)
RT = CAP // P        # 4 tiles per region
NSLOT = 3 * CAP      # slots per expert group
GT = NSLOT // P      # 12 slot-tiles per group
NW = NSLOT // 16     # idx columns (wrapped in 16 partitions)
CW = NW // 3         # idx columns per region

_BF16 = ml_dtypes.bfloat16
_F8 = ml_dtypes.float8_e4m3

_prog_cache = {}


def _build_program(cfg):
    """cfg = (router_bias_nz, syn_bias_nz, len_bias_nz, sem_bias_nz, cls_bias_nz)"""
    import concourse.bass as bass
    import concourse.tile as tile
    from concourse import bacc, mybir

    rb_nz, synb_nz, lenb_nz, semb_nz, clsb_nz = cfg
    f32 = mybir.dt.float32
    i32 = mybir.dt.int32
    i16 = mybir.dt.int16
    u32 = mybir.dt.uint32
    bf16 = mybir.dt.bfloat16
    fp8 = mybir.dt.float8e4
    AF = mybir.ActivationFunctionType
    ALU = mybir.AluOpType
    AX = mybir.AxisListType
    DR = mybir.MatmulPerfMode.DoubleRow
    ts = bass.ts

    nc = bacc.Bacc("TRN2", target_bir_lowering=False, debug=False,
                   dynamic_dma_scratch_size=24576)

    # ---- DRAM I/O ----
    hsb_d = nc.dram_tensor("hsb", [S, H], bf16, kind="ExternalInput").ap()
    hbm_d = nc.dram_tensor("hbm", [KT * 3 * P, S], bf16, kind="ExternalInput").ap()
    hs8T_d = nc.dram_tensor("hs8T", [H, S], fp8, kind="ExternalInput").ap()
    hsrT_d = nc.dram_tensor("hsrT", [H, S], fp8, kind="ExternalInput").ap()
    hs8r_d = nc.dram_tensor("hs8r", [S, H], fp8, kind="ExternalInput").ap()
    hsrr_d = nc.dram_tensor("hsrr", [S, H], fp8, kind="ExternalInput").ap()
    adjT_d = nc.dram_tensor("adjT", [S, S], fp8, kind="ExternalInput").ap()
    rw1_d = nc.dram_tensor("rw1", [H, 7], bf16, kind="ExternalInput").ap()
    rw2_d = nc.dram_tensor("rw2", [H, 7], bf16, kind="ExternalInput").ap()
    wg1_d = nc.dram_tensor("wg1", [H, H], fp8, kind="ExternalInput").ap()
    wg2_d = nc.dram_tensor("wg2", [H, H], fp8, kind="ExternalInput").ap()
    iotaw_d = nc.dram_tensor("iotaw", [16, S // 16], i32, kind="ExternalInput").ap()
    wexp_d = []  # merged (w8; wr) rows per expert: len, sem0-2, syn0-2
    for nm in ["len", "sem0", "sem1", "sem2", "syn0", "syn1", "syn2"]:
        wexp_d.append(
            nc.dram_tensor(f"w{nm}p", [2 * H, H], fp8, kind="ExternalInput").ap())
    wcls_d = nc.dram_tensor("wcls", [H, 2], bf16, kind="ExternalInput").ap()
    br_d = nc.dram_tensor("br", [1, 7], f32, kind="ExternalInput").ap() if rb_nz else None
    bsyn_d = nc.dram_tensor("bsyn", [3, H], f32, kind="ExternalInput").ap() if synb_nz else None
    blen_d = nc.dram_tensor("blen", [1, H], f32, kind="ExternalInput").ap() if lenb_nz else None
    bsem_d = nc.dram_tensor("bsem", [3, H], f32, kind="ExternalInput").ap() if semb_nz else None
    bcls_d = nc.dram_tensor("bcls", [1, 2], f32, kind="ExternalInput").ap() if clsb_nz else None
    outC_d = nc.dram_tensor("outC", [P, ST, 2], f32, kind="ExternalOutput").ap()
    outB_d = nc.dram_tensor("outB", [P, GT, 2], f32, kind="ExternalOutput").ap()
    outA_d = nc.dram_tensor("outA", [P, GT, 2], f32, kind="ExternalOutput").ap()
    idxsyn_d = nc.dram_tensor("idxsyn", [16, NW], i16, kind="ExternalOutput").ap()
    idxsem_d = nc.dram_tensor("idxsem", [16, NW], i16, kind="ExternalOutput").ap()
    cnts_d = nc.dram_tensor("cnts", [1, 8], u32, kind="ExternalOutput").ap()

    hs_r = hsb_d.rearrange("(a p) h -> p a h", p=P)
    hbm_r = hbm_d.rearrange("(k t p) s -> p k t s", p=P, t=3)
    hs8T_r = hs8T_d.rearrange("(k p) s -> p k s", p=P)
    hsrT_r = hsrT_d.rearrange("(k p) s -> p k s", p=P)
    adjT_r = adjT_d.rearrange("(t p) s -> p t s", p=P)
    rw1_r = rw1_d.rearrange("(k p) e -> p k e", p=P)
    rw2_r = rw2_d.rearrange("(k p) e -> p k e", p=P)
    wcls_r = wcls_d.rearrange("(k p) c -> p k c", p=P)
    wg1_r = wg1_d.rearrange("(k p) d -> p k d", p=P)
    wg2_r = wg2_d.rearrange("(k p) d -> p k d", p=P)
    wexp_r = [w.rearrange("(k p) d -> p k d", p=P) for w in wexp_d]

    with tile.TileContext(nc) as tc, ExitStack() as ctx:
        # ---- pools ----
        const = ctx.enter_context(tc.tile_pool(name="const", bufs=1))
        hspool = ctx.enter_context(tc.tile_pool(name="hspool", bufs=1))
        f8pool = ctx.enter_context(tc.tile_pool(name="f8pool", bufs=1))
        wpool = ctx.enter_context(tc.tile_pool(name="wpool", bufs=3))
        fpool = ctx.enter_context(tc.tile_pool(name="fpool", bufs=2))
        gpool = ctx.enter_context(tc.tile_pool(name="gpool", bufs=1))
        stage = ctx.enter_context(tc.tile_pool(name="stage", bufs=2))
        small = ctx.enter_context(tc.tile_pool(name="small", bufs=2))
        dram = ctx.enter_context(tc.tile_pool(name="dram", bufs=1, space="DRAM"))
        acc = ctx.enter_context(tc.tile_pool(name="acc", bufs=4, space="PSUM"))
        spsum = ctx.enter_context(tc.tile_pool(name="spsum", bufs=2, space="PSUM"))

        # ---- DRAM scratch (dependency-tracked pool tiles) ----
        shs = dram.tile([S, H], bf16, tag="shs")
        ctab = dram.tile([S, 64], f32, tag="ctab")
        shs_r = shs[:].rearrange("(a p) h -> p a h", p=P)

        # ---- constants (gpsimd DMA queue, parallel to sync queue) ----
        rw1_sb = const.tile([P, KT, 7], bf16, tag="rw1")
        nc.gpsimd.dma_start(rw1_sb[:], rw1_r)
        rw2_sb = const.tile([P, KT, 7], bf16, tag="rw2")
        nc.gpsimd.dma_start(rw2_sb[:], rw2_r)
        wcls_sb = const.tile([P, KT, 2], bf16, tag="wcls")
        nc.gpsimd.dma_start(wcls_sb[:], wcls_r)
        iotaw = const.tile([16, S // 16], i32, tag="iotaw")
        nc.gpsimd.dma_start(iotaw[:], iotaw_d)
        ones_row = None
        if any(x is not None for x in (br_d, bsyn_d, blen_d, bsem_d, bcls_d)):
            ones_row = const.tile([1, P], f32, tag="ones")
            nc.vector.memset(ones_row[:], 1.0)

        def bias_row(dram_ap, n, tag):
            t = const.tile([1, n], f32, tag=tag)
            nc.gpsimd.dma_start(t[:], dram_ap)
            return t

        br_sb = bias_row(br_d, 7, "br") if br_d is not None else None
        blen_sb = bias_row(blen_d, H, "blen") if blen_d is not None else None
        bsem_sb = ([bias_row(bsem_d[e : e + 1, :], H, f"bsem{e}") for e in range(3)]
                   if bsem_d is not None else None)
        bsyn_sb = ([bias_row(bsyn_d[e : e + 1, :], H, f"bsyn{e}") for e in range(3)]
                   if bsyn_d is not None else None)

        # ---- persistent SBUF tensors ----
        hs_all = hspool.tile([P, ST, H], bf16, tag="hs")      # hs -> resid -> fusedC
        hs8T = f8pool.tile([P, KT, S], fp8, tag="hs8T")
        hsrT = f8pool.tile([P, KT, S], fp8, tag="hsrT")
        adjT = f8pool.tile([P, TT, S], fp8, tag="adjT")       # 256 * Anorm^T

        # ---- weight chunk streaming ([P, KT, NCH] slices, consumption order) --
        # all wpool tiles (weight chunks + router hb slices) alternate between
        # the two rotating tag slots in issue order == consumption order
        _tag_ctr = [0]

        def next_tag():
            _tag_ctr[0] += 1
            return "w8" if _tag_ctr[0] % 2 else "wr"

        def load_wc(wdram_r, n, tag=None):
            # GCN single chunks ride the small rotating tags
            wt = wpool.tile([P, KT, WCH], fp8, tag=next_tag(), name=f"wc{_tag_ctr[0]}", bufs=2)
            nc.sync.dma_start(wt[:], wdram_r[:, :, ts(n, WCH)])
            return wt

        def load_wpair(wdram_r, n):
            # merged (w8; wr) chunk: one DMA per pass
            _tag_ctr[0] += 1
            wt = wpool.tile([P, 2 * KT, WCH], fp8, tag="wc", name=f"wp{_tag_ctr[0]}", bufs=2)
            nc.sync.dma_start(wt[:], wdram_r[:, :, ts(n, WCH)])
            return wt

        # router hb k-slices stream through a small pool; DMAs are interleaved
        # with the GCN chunk loads and consumed by router k-blocks interleaved
        # with the GCN matmul phases (so neither queue stalls the other)
        hb_sl = [None] * KT

        def load_hb(k):
            sl = wpool.tile([P, 3, S], bf16, tag=next_tag(), name=f"hb_{k}", bufs=2)
            nc.sync.dma_start(sl[:], hbm_r[:, k, :, :])
            hb_sl[k] = sl

        # GCN1 chunk 0 + full hs8T first (needed for the first psum group)
        nc.sync.dma_start(hs8T[:, 0:6, :], hs8T_r[:, 0:6, :])
        wg1_c0 = load_wc(wg1_r, 0)
        nc.sync.dma_start(hs8T[:, 6:12, :], hs8T_r[:, 6:12, :])
        wg1_c = [wg1_c0] + [load_wc(wg1_r, n) for n in range(1, WNN)]
        nc.sync.dma_start(adjT[:], adjT_r)
        wg2_c = [load_wc(wg2_r, n) for n in range(WNN)]
        for m in range(3):
            nc.sync.dma_start(hs_all[:, m, :], hs_r[:, m, :])
        load_hb(0)
        load_hb(1)
        load_hb(2)
        load_hb(3)
        nc.sync.dma_start(hs_all[:, 3, :], hs_r[:, 3, :])
        load_hb(4)
        load_hb(5)
        nc.sync.dma_start(hs_all[:, 4, :], hs_r[:, 4, :])
        load_hb(6)
        load_hb(7)
        nc.sync.dma_start(hs_all[:, 5, :], hs_r[:, 5, :])
        load_hb(8)
        load_hb(9)
        nc.sync.dma_start(hs_all[:, 6, :], hs_r[:, 6, :])
        load_hb(10)
        load_hb(11)
        nc.sync.dma_start(hs_all[:, 7, :], hs_r[:, 7, :])
        nc.sync.dma_start(hsrT[:], hsrT_r)
        # expert weight chunks: len, sem0-2, syn0-2; (w8, wr) pairs per n-chunk
        wexp_c = []
        for ei in range(7):
            wexp_c.append([load_wpair(wexp_r[ei], n) for n in range(WNN)])

        # ---- router k-blocks (fp32-exact logits from bf16 triple/pair split) --
        logit = small.tile([P, ST, 7], f32, tag="logit", bufs=1)
        nc.vector.memset(logit[:], 0.0)

        def router_k(k):
            rlog = spsum.tile([P, ST, 7], f32, tag="sp")
            hb = hb_sl[k]
            terms = ((0, rw1_sb), (0, rw2_sb), (1, rw1_sb), (1, rw2_sb),
                     (2, rw1_sb))
            for m in range(ST):
                for t_i, (tt, rwt) in enumerate(terms):
                    nc.tensor.matmul(rlog[:, m, :], hb[:, tt, ts(m, P)],
                                     rwt[:, k, :], start=(t_i == 0),
                                     stop=(t_i == len(terms) - 1))
            nc.vector.tensor_add(logit[:], logit[:], rlog[:])

        # ---- GCN: S1 = hs8 @ W1q (fp8 DR), evict /32 -> fp8 [s, d] ----
        s_sb = f8pool.tile([P, ST, H], fp8, tag="s12")
        for n in range(WNN):
            for m in range(ST):
                ps = acc.tile([P, WCH], f32, tag="acc")
                for j in range(KD):
                    nc.tensor.matmul(ps[:], hs8T[:, 2 * j : 2 * j + 2, ts(m, P)],
                                     wg1_c[n][:, 2 * j : 2 * j + 2, :],
                                     start=(j == 0), stop=(j == KD - 1), perf_mode=DR)
                nc.scalar.activation(s_sb[:, m, ts(n, WCH)], ps[:], AF.Copy,
                                     scale=1.0 / WS)

        # ---- x1T = relu(Anorm @ S1)^T via lhsT=S1: store 32*relu(x1) fp8 ----
        x1T = f8pool.tile([P, KT, S], fp8, tag="x1T")
        for dt_i in range(KT):
            for sc in range(2):
                ps = acc.tile([P, NCH], f32, tag="acc")
                for j in range(TD):
                    nc.tensor.matmul(ps[:], s_sb[:, 2 * j : 2 * j + 2, ts(dt_i, P)],
                                     adjT[:, 2 * j : 2 * j + 2, ts(sc, NCH)],
                                     start=(j == 0), stop=(j == TD - 1), perf_mode=DR)
                nc.scalar.activation(x1T[:, dt_i, ts(sc, NCH)], ps[:], AF.Relu,
                                     scale=WS / ASC)

        # ---- S2 = (32 x1) @ W2q: store 32*S2 in fp8 ----
        s2_sb = f8pool.tile([P, ST, H], fp8, tag="s12")
        for n in range(WNN):
            for m in range(ST):
                ps = acc.tile([P, WCH], f32, tag="acc")
                for j in range(KD):
                    nc.tensor.matmul(ps[:], x1T[:, 2 * j : 2 * j + 2, ts(m, P)],
                                     wg2_c[n][:, 2 * j : 2 * j + 2, :],
                                     start=(j == 0), stop=(j == KD - 1), perf_mode=DR)
                nc.scalar.activation(s2_sb[:, m, ts(n, WCH)], ps[:], AF.Copy,
                                     scale=1.0 / WS)

        # ---- residual + LayerNorm -> sh (bf16) ----
        sh_t = [None] * ST

        def do_ln(m):
            stats = small.tile([P, NN, 6], bf16, tag="stats", bufs=1)
            for c in range(NN):
                nc.vector.bn_stats(stats[:, c, :], hs_all[:, m, ts(c, NCH)])
            mv = small.tile([P, 2], f32, tag="mv", bufs=1)
            nc.vector.bn_aggr(mv[:], stats[:])
            # rsqrt via bit-trick seed + Newton step on DVE (keeps Sqrt off Act)
            veps = small.tile([P, 1], f32, tag="veps", bufs=1)
            nc.vector.tensor_scalar(out=veps[:], in0=mv[:, 1:2], scalar1=EPS,
                                    scalar2=None, op0=ALU.add)
            rsd_i = small.tile([P, 1], i32, tag="rsdi", bufs=1)
            nc.vector.tensor_scalar(out=rsd_i[:], in0=veps[:].bitcast(i32),
                                    scalar1=1, scalar2=None,
                                    op0=ALU.logical_shift_right)
            nc.vector.tensor_scalar(out=rsd_i[:], in0=rsd_i[:], scalar1=-1,
                                    scalar2=0x5F3759DF, op0=ALU.mult, op1=ALU.add)
            rstd = rsd_i[:].bitcast(f32)
            nwt = small.tile([P, 1], f32, tag="nwt", bufs=1)
            nc.vector.tensor_mul(nwt[:], rstd, rstd)
            nc.vector.tensor_mul(nwt[:], nwt[:], veps[:])
            nc.vector.tensor_scalar(out=nwt[:], in0=nwt[:], scalar1=-0.5,
                                    scalar2=1.5, op0=ALU.mult, op1=ALU.add)
            nc.vector.tensor_mul(rstd, rstd, nwt[:])
            nmr = small.tile([P, 1], f32, tag="nmr", bufs=1)
            nc.vector.tensor_mul(nmr[:], mv[:, 0:1], rstd)
            nc.vector.tensor_scalar_mul(nmr[:], nmr[:], -1.0)
            sh = stage.tile([P, H], bf16, tag="shm", bufs=2)
            nc.scalar.activation(sh[:], hs_all[:, m, :], AF.Identity,
                                 bias=nmr[:], scale=rstd)
            nc.gpsimd.dma_start(shs_r[:, m, :], sh[:])
            sh_t[m] = sh

        # ---- x2: resid += relu(psum)/8192 (residual adds on DVE) ----
        for m in range(ST):
            for n in range(NN):
                ps = acc.tile([P, NCH], f32, tag="acc")
                for j in range(TD):
                    nc.tensor.matmul(ps[:], adjT[:, 2 * j : 2 * j + 2, ts(m, P)],
                                     s2_sb[:, 2 * j : 2 * j + 2, ts(n, NCH)],
                                     start=(j == 0), stop=(j == TD - 1), perf_mode=DR)
                g = stage.tile([P, NCH], bf16, tag="hTf", bufs=3)
                nc.scalar.activation(g[:], ps[:], AF.Relu, scale=1.0 / (ASC * WS))
                nc.vector.tensor_add(hs_all[:, m, ts(n, NCH)],
                                     hs_all[:, m, ts(n, NCH)], g[:])
                gidx = m * NN + n
                if gidx % 2 == 0 and gidx // 2 < KT:
                    router_k(gidx // 2)
                if n == NN - 1:
                    do_ln(m)

        if br_sb is not None:
            rlog = spsum.tile([P, ST, 7], f32, tag="sp")
            for m in range(ST):
                nc.tensor.matmul(rlog[:, m, :], ones_row[:], br_sb[:],
                                 start=True, stop=True)
            nc.vector.tensor_add(logit[:], logit[:], rlog[:])

        # ---- router math: group softmax ratios + top-1 coefficients ----
        # coef table layout (64 f32 per token): 0=clen, 1..3=csyn, 4..6=csem
        coef_sb = small.tile([P, ST, 64], f32, tag="ctabs", bufs=1)
        nc.vector.memset(coef_sb[:], 0.0)
        e_sb = small.tile([P, ST, 7], f32, tag="esb", bufs=1)
        nc.scalar.activation(e_sb[:], logit[:], AF.Exp)
        gdum = small.tile([1, 4], f32, tag="gdum", bufs=1)
        nc.scalar.activation(gdum[:], e_sb[0:1, 0, 0:4], AF.Gelu)
        syn_e = small.tile([P, ST], f32, tag="syn_e", bufs=1)
        nc.vector.tensor_reduce(syn_e[:], e_sb[:, :, 0:3], axis=AX.X, op=ALU.max)
        sem_e = small.tile([P, ST], f32, tag="sem_e", bufs=1)
        nc.vector.tensor_reduce(sem_e[:], e_sb[:, :, 4:7], axis=AX.X, op=ALU.max)
        rden = small.tile([P, ST], f32, tag="rden", bufs=1)
        nc.vector.tensor_add(rden[:], syn_e[:], sem_e[:])
        nc.vector.tensor_add(rden[:], rden[:], e_sb[:, :, 3])
        nc.vector.reciprocal(rden[:], rden[:])
        nc.vector.tensor_mul(coef_sb[:, :, 0], e_sb[:, :, 3], rden[:])

        cls_f = [None, None]  # f32 class vecs: [syn, sem]

        def group_coefs(gi, ccol, base, w_e):
            """coef cols ccol..ccol+2 = rden * w_e * mask_e; class vec = first-max
            argmax over logit columns base..base+2 (matches jnp tie-breaking)."""
            l0, l1, l2 = (logit[:, :, base + i] for i in range(3))
            s0 = small.tile([P, ST], f32, tag="s0", bufs=1)
            ge02 = small.tile([P, ST], f32, tag="ge02", bufs=1)
            nc.vector.tensor_tensor(out=s0[:], in0=l0, in1=l1, op=ALU.is_ge)
            nc.vector.tensor_tensor(out=ge02[:], in0=l0, in1=l2, op=ALU.is_ge)
            nc.vector.tensor_mul(s0[:], s0[:], ge02[:])
            s1 = small.tile([P, ST], f32, tag="s1", bufs=1)
            ge12 = small.tile([P, ST], f32, tag="ge12", bufs=1)
            nc.vector.tensor_tensor(out=ge12[:], in0=l1, in1=l2, op=ALU.is_ge)
            nc.vector.tensor_mul(s1[:], s0[:], ge12[:])
            nc.vector.tensor_tensor(out=s1[:], in0=ge12[:], in1=s1[:], op=ALU.subtract)
            s2 = small.tile([P, ST], f32, tag="s2", bufs=1)
            nc.vector.tensor_add(s2[:], s0[:], s1[:])
            nc.vector.tensor_scalar(out=s2[:], in0=s2[:], scalar1=-1.0, scalar2=1.0,
                                    op0=ALU.mult, op1=ALU.add)
            for e, sm in enumerate((s0, s1, s2)):
                nc.vector.tensor_mul(coef_sb[:, :, ccol + e], sm[:], w_e)
                nc.vector.tensor_mul(coef_sb[:, :, ccol + e],
                                     coef_sb[:, :, ccol + e], rden[:])
            cg = small.tile([P, ST], f32, tag=f"clsv{gi}", bufs=1)
            nc.vector.tensor_scalar(out=cg[:], in0=s2[:], scalar1=2.0, scalar2=None,
                                    op0=ALU.mult)
            nc.vector.tensor_add(cg[:], cg[:], s1[:])
            cls_f[gi] = cg

        group_coefs(0, 1, 0, syn_e[:])
        group_coefs(1, 4, 4, sem_e[:])
        nc.gpsimd.dma_start(ctab[:].rearrange("(a p) c -> p a c", p=P), coef_sb[:])

        # ---- counting sort per group: wrapped class -> sparse_gather lists ----
        cnts_sb = small.tile([1, 8], u32, tag="cnts", bufs=1)
        nc.vector.memset(cnts_sb[:], 0)
        idx_tiles = []
        for gi in range(2):  # 0=syn, 1=sem
            cg_i = small.tile([P, ST], i32, tag="cgi", bufs=1)
            nc.vector.tensor_copy(cg_i[:], cls_f[gi][:])
            clsw = small.tile([16, ST, 8], i32, tag="clsw", bufs=1)
            for r in range(8):
                nc.gpsimd.dma_start(clsw[:, :, r], cg_i[r * 16:(r + 1) * 16, :])
            clsw_f = clsw[:].rearrange("q a r -> q (a r)")
            arr = small.tile([16, 3, S // 16], f32, tag="arr", bufs=1)
            msk = small.tile([16, S // 16], i32, tag="msk", bufs=1)
            iop = small.tile([16, S // 16], i32, tag="iop", bufs=1)
            for c in range(3):
                nc.vector.tensor_scalar(out=msk[:], in0=clsw_f, scalar1=c,
                                        scalar2=None, op0=ALU.is_equal)
                nc.vector.tensor_scalar(out=iop[:], in0=iotaw[:], scalar1=1,
                                        scalar2=None, op0=ALU.add)
                nc.vector.tensor_tensor(out=iop[:], in0=msk[:], in1=iop[:],
                                        op=ALU.mult)
                nc.vector.tensor_scalar(out=arr[:, c, :], in0=iop[:], scalar1=-1,
                                        scalar2=None, op0=ALU.add)
            glist = small.tile([16, 3, S // 16], f32, tag="gl", bufs=1)
            for c in range(3):
                nc.gpsimd.sparse_gather(
                    glist[:, c, :], arr[:, c, :],
                    num_found=cnts_sb[0:1, 3 * gi + c : 3 * gi + c + 1])
            gmax = small.tile([16, 3, CW], f32, tag="gmax", bufs=1)
            nc.vector.tensor_scalar(out=gmax[:], in0=glist[:, :, 0:CW],
                                    scalar1=0.0, scalar2=float(S - 1),
                                    op0=ALU.max, op1=ALU.min)
            idxs = gpool.tile([P, NW], i16, tag=f"idx{gi}", name=f"idx{gi}")
            nc.vector.tensor_copy(idxs[0:16, :], gmax[:].rearrange("q c n -> q (c n)"))
            for g in range(1, 8):
                nc.gpsimd.dma_start(idxs[g * 16:(g + 1) * 16, :], idxs[0:16, :])
            idx_tiles.append(idxs)
            nc.sync.dma_start(idxsyn_d if gi == 0 else idxsem_d, idxs[0:16, :])
        nc.sync.dma_start(cnts_d, cnts_sb[:])
        idx_syn, idx_sem = idx_tiles

        # ---- indirect gathers: region x tensors + coef tables ----
        # gathered layout per region: [p, f, s, b] with h = 256f + 2p + b
        def gather_x(src_dram, idxs, tags, pool):
            tiles = []
            for c in range(3):
                t = pool.tile([P, 6, CAP, 2], fp8, tag=tags[c], name=f"{tags[c]}x")
                gv = t[:].rearrange("p f s b -> p (f s b)").rearrange(
                    "p (t n) -> p t n", t=KT)
                nc.gpsimd.dma_gather(gv, src_dram, idxs[:, c * CW:(c + 1) * CW],
                                     CAP, CAP, H, elem_step=H, transpose=True)
                tiles.append(t)
            return tiles

        def gather_coef(idxs, tag):
            t = gpool.tile([P, GT, 8], f32, tag=tag, name=tag)
            for c in range(3):
                cscr = gpool.tile([P, RT, 64], f32, tag="cscr", name=f"cs{tag}{c}")
                nc.gpsimd.dma_gather(cscr[:], ctab[:],
                                     idxs[:, c * CW:(c + 1) * CW],
                                     CAP, CAP, 64, elem_step=64, transpose=False)
                nc.vector.tensor_copy(t[:, c * RT:(c + 1) * RT, :],
                                      cscr[:, :, 0:8])
            return t

        xg8_sem = gather_x(hs8r_d, idx_sem, ["s12", "x1T", "adjT"], f8pool)
        xgr_sem = gather_x(hsrr_d, idx_sem, ["xgb0", "xgb1", "xgb2"], gpool)
        coefp_sem = gather_coef(idx_sem, "cpsem")
        coefp_syn = gather_coef(idx_syn, "cpsyn")


        # ---- len expert (dense, original token order) into hs_all ----

        x8, xr = hs8T, hsrT
        for n in range(WNN):
            wp = wexp_c[0][n]
            for m in range(ST):
                ps = acc.tile([P, WCH], f32, tag="acc")
                for t_i, (xx, ko) in enumerate(((x8, 0), (xr, 0), (x8, KT))):
                    for j in range(KD):
                        last = (t_i == 2 and j == KD - 1 and blen_sb is None)
                        nc.tensor.matmul(
                            ps[:], xx[:, 2 * j : 2 * j + 2, ts(m, P)],
                            wp[:, ko + 2 * j : ko + 2 * j + 2, :],
                            start=(t_i == 0 and j == 0), stop=last, perf_mode=DR)
                if blen_sb is not None:
                    nc.tensor.matmul(ps[:], ones_row[:], blen_sb[:, ts(n, WCH)],
                                     start=False, stop=True)
                g = stage.tile([P, WCH], bf16, tag="hTf", bufs=3)
                nc.scalar.activation(g[:], ps[:], AF.Gelu, scale=1.0 / WS)
                nc.vector.tensor_scalar_mul(hs_all[:, m, ts(n, WCH)], g[:],
                                            coef_sb[:, m, 0:1])

        # ---- cls tail helpers (linear head applied per partial output) ----
        out_sbC = small.tile([P, ST, 2], f32, tag="outC", bufs=1)
        out_sbB = small.tile([P, GT, 2], f32, tag="outB", bufs=1)
        out_sbA = small.tile([P, GT, 2], f32, tag="outA", bufs=1)
        bcls_sb = bias_row(bcls_d, 2, "bcls") if bcls_d is not None else None

        def make_tail(out_sb, with_bias):
            fuT_t = {}

            def pre(src_ap, m):
                fuT = stage.tile([P, KT, P], bf16, tag="fuT", bufs=4)
                nc.scalar.dma_start_transpose(fuT[:], src_ap)
                fuT_t[m] = fuT

            def cls(m):
                fuT = fuT_t.pop(m)
                cps = spsum.tile([P, 2], f32, tag="cls")
                for k in range(KT):
                    last = (k == KT - 1) and not (with_bias and bcls_sb is not None)
                    nc.tensor.matmul(cps[:], fuT[:, k, :], wcls_sb[:, k, :],
                                     start=(k == 0), stop=last)
                if with_bias and bcls_sb is not None:
                    nc.tensor.matmul(cps[:], ones_row[:], bcls_sb[:],
                                     start=False, stop=True)
                nc.vector.tensor_copy(out_sb[:, m, :], cps[:])

            return pre, cls

        preC, clsC = make_tail(out_sbC, True)
        preB, clsB = make_tail(out_sbB, False)
        preA, clsA = make_tail(out_sbA, False)

        # ---- region experts: region c of a group evaluates expert c ----
        # (per-slot coefs are zero for other-class tokens / dead pad slots)
        def region_expert(xg8, xgr, wcs, coefp, ccol, c, bias_sb, after_tile,
                          il=True):
            fb = fpool.tile([P, RT, H], bf16, tag="fus", name=f"fus{ccol}_{c}")
            for n in range(WNN):
                wp = wcs[n]
                for mm in range(RT):
                    ps = acc.tile([P, WCH], f32, tag="acc")
                    k = 0
                    for xx, ko in ((xg8, 0), (xgr, 0), (xg8, KT)):
                        for jj in range(KD):
                            k += 1
                            last = (k == 18 and bias_sb is None)
                            if il:
                                bb, f0 = jj // 3, 2 * (jj % 3)
                                lhsT = xx[:, f0:f0 + 2, ts(mm, P), bb]
                                rhs = wp[:, ko + 6 * bb + f0 : ko + 6 * bb + f0 + 2, :]
                            else:
                                lhsT = xx[:, 2 * jj:2 * jj + 2, ts(mm, P)]
                                rhs = wp[:, ko + 2 * jj : ko + 2 * jj + 2, :]
                            nc.tensor.matmul(ps[:], lhsT, rhs,
                                             start=(k == 1), stop=last,
                                             perf_mode=DR)
                    if bias_sb is not None:
                        nc.tensor.matmul(ps[:], ones_row[:], bias_sb[:, ts(n, WCH)],
                                         start=False, stop=True)
                    g = stage.tile([P, WCH], bf16, tag="hTf", bufs=3)
                    nc.scalar.activation(g[:], ps[:], AF.Gelu, scale=1.0 / WS)
                    nc.vector.tensor_scalar_mul(
                        fb[:, mm, ts(n, WCH)], g[:],
                        coefp[:, c * RT + mm, ccol:ccol + 1])
                    after_tile(fb, n * RT + mm, c * RT + mm)

        # sem phase: fusedB region tails + fusedC (hs_all) tails, with cls
        # trailing its transpose by two hooks to hide the fuT DMA latency
        # pending cls calls drain in the NEXT region's early steps, giving
        # each fuT transpose a full region of lead time
        pending_cls = []

        def drain_cls(k=1):
            for _ in range(k):
                if pending_cls:
                    pending_cls.pop(0)()

        def sem_after(fb, si, gm, creg):
            if creg == 0 and si < ST:
                preC(hs_all[:, si, :], si)
                pending_cls.append(lambda m=si: clsC(m))
            if si < (WNN - 1) * RT:
                if si >= 2:
                    drain_cls()
                return
            mm = gm % RT
            preB(fb[:, mm, :], gm)
            pending_cls.append(lambda gm=gm: clsB(gm))
            if gm == ST - 1:
                pending_cls.append(
                    lambda: nc.sync.dma_start(outC_d, out_sbC[:]))

        # syn-group gather+split: bf16 transpose-gather of shared rows into a
        # temp (standard k-tile layout), then split to the fp8 pair on-chip;
        # issued between sem regions so each region's WAR deps resolve in turn
        xg8_syn = [None] * 3
        xgr_syn = [None] * 3
        syn_tmp = [None] * 3

        def syn_gather(c):
            tmp = f8pool.tile([P, KT, CAP], bf16, tag="hs8T" if c % 2 == 0 else "hsrT",
                              name=f"sgt{c}")
            nc.gpsimd.dma_gather(tmp[:], shs[:], idx_syn[:, c * CW:(c + 1) * CW],
                                 CAP, CAP, H, elem_step=H, transpose=True)
            syn_tmp[c] = tmp
            x8 = f8pool.tile([P, KT, CAP], fp8, tag=("s12", "x1T", "adjT")[c],
                             name=f"sg8{c}")
            xr = gpool.tile([P, KT, CAP], fp8, tag=f"xgb{c}", name=f"sgr{c}")
            xg8_syn[c] = x8
            xgr_syn[c] = xr

        def split_views(c, h):
            tmp3 = syn_tmp[c][:].rearrange("p k (q n) -> p (k q) n", q=RT)
            x83 = xg8_syn[c][:].rearrange("p k (q n) -> p (k q) n", q=RT)
            xr3 = xgr_syn[c][:].rearrange("p k (q n) -> p (k q) n", q=RT)
            part = slice(h * 4, h * 4 + 4)
            return tmp3[:, part, :], x83[:, part, :], xr3[:, part, :]

        def split_copy(c, h):
            tmp_p, x8_p, _ = split_views(c, h)
            nc.scalar.activation(x8_p, tmp_p, AF.Copy)

        def split_sub(c, h):
            tmp_p, x8_p, xr_p = split_views(c, h)
            nc.vector.tensor_tensor(out=xr_p, in0=tmp_p, in1=x8_p,
                                    op=ALU.subtract)

        for c in range(3):
            # interleave split pieces of region c-1's syn input into this
            # region's eval steps (slots freed at region c-1's end)
            cc = c - 1  # split pieces for the previous region's syn input

            def hook(fb, si, gm, cc=cc):
                sem_after(fb, si, gm, cc + 1)
                if cc < 0:
                    return
                if si < KT:
                    split_copy(cc, si)
                if 2 <= si < KT + 2:
                    split_sub(cc, si - 2)

            region_expert(xg8_sem[c], xgr_sem[c], wexp_c[1 + c], coefp_sem,
                          4 + c, c, bsem_sb[c] if bsem_sb else None, hook)
            syn_gather(c)

        def syn_after(fa, si, gm):
            if si < (WNN - 1) * RT:
                drain_cls()
                return
            mm = gm % RT
            preA(fa[:, mm, :], gm)
            pending_cls.append(lambda gm=gm: clsA(gm))
            if gm == GT - 1:
                pending_cls.append(
                    lambda: nc.sync.dma_start(outB_d, out_sbB[:]))

        for c in range(3):
            def hook(fa, si, gm, c=c):
                syn_after(fa, si, gm)
                if c != 0:
                    return
                if si < KT:
                    split_copy(2, si)
                if 2 <= si < KT + 2:
                    split_sub(2, si - 2)

            region_expert(xg8_syn[c], xgr_syn[c], wexp_c[4 + c], coefp_syn,
                          1 + c, c, bsyn_sb[c] if bsyn_sb else None, hook,
                          il=False)
        drain_cls(len(pending_cls))
        nc.sync.dma_start(outA_d, out_sbA[:])

    nc.compile()
    return nc


def _get_program(cfg):
    if cfg not in _prog_cache:
        _prog_cache[cfg] = _build_program(cfg)
    return _prog_cache[cfg]


def _fp8_pair(w):
    """w -> (q8(32w), q8(32w - float(q8(32w)))) as contiguous fp8 arrays."""
    ws = (WS * w).astype(np.float32)
    w8 = ws.astype(_F8)
    wr = (ws - w8.astype(np.float32)).astype(_F8)
    return np.ascontiguousarray(w8), np.ascontiguousarray(wr)


# dma_gather transposes fp8 at u16 granularity: gathered[p, f, s, b] holds
# h = 256f + 2p + b; sem/syn weight rows are pre-permuted to match, with
# k-tile t = 6b + f containing rows 256f + 2p + b.
_IL_ROWS = np.empty(H, dtype=np.int64)
for _b in range(2):
    for _f in range(6):
        _IL_ROWS[(6 * _b + _f) * P:(6 * _b + _f + 1) * P] = \
            256 * _f + 2 * np.arange(P) + _b


def _fp8_pair_il(w):
    w8, wr = _fp8_pair(w)
    return (np.ascontiguousarray(w8[_IL_ROWS, :]),
            np.ascontiguousarray(wr[_IL_ROWS, :]))


def kernel(**inputs):
    from concourse import bass_utils

    hs = np.asarray(inputs["hidden_states"], dtype=np.float32)
    adj = np.asarray(inputs["adj_matrix"], dtype=np.float32)
    seq_lengths = np.asarray(inputs["seq_lengths"])
    router_w = np.asarray(inputs["router_w"], dtype=np.float32)
    router_b = np.asarray(inputs["router_b"], dtype=np.float32)
    gcn1_w = np.asarray(inputs["gcn1_w"], dtype=np.float32)
    gcn2_w = np.asarray(inputs["gcn2_w"], dtype=np.float32)
    ln_g = np.asarray(inputs["ln_g"], dtype=np.float32)
    ln_b = np.asarray(inputs["ln_b"], dtype=np.float32)
    syn_w = np.asarray(inputs["syn_w"], dtype=np.float32)
    syn_b = np.asarray(inputs["syn_b"], dtype=np.float32)
    len_short_w = np.asarray(inputs["len_short_w"], dtype=np.float32)
    len_short_b = np.asarray(inputs["len_short_b"], dtype=np.float32)
    len_long_w = np.asarray(inputs["len_long_w"], dtype=np.float32)
    len_long_b = np.asarray(inputs["len_long_b"], dtype=np.float32)
    sem_w = np.asarray(inputs["sem_w"], dtype=np.float32)
    sem_b = np.asarray(inputs["sem_b"], dtype=np.float32)
    cls_w = np.asarray(inputs["cls_w"], dtype=np.float32)
    cls_b = np.asarray(inputs["cls_b"], dtype=np.float32)

    # fold LN affine into syn expert weights
    syn_w_f = (ln_g[None, :, None] * syn_w).astype(np.float32)
    syn_b_f = (syn_b + np.einsum("h,ehd->ed", ln_b, syn_w)).astype(np.float32)

    is_short = seq_lengths <= THRESHOLD

    cfg = (
        bool(np.any(router_b != 0)),
        bool(np.any(syn_b_f != 0)),
        bool(np.any(len_short_b != 0) or np.any(len_long_b != 0)),
        bool(np.any(sem_b != 0)),
        bool(np.any(cls_b != 0)),
    )
    nc = _get_program(cfg)

    wg1_8, _ = _fp8_pair(gcn1_w)
    wg2_8, _ = _fp8_pair(gcn2_w)

    def _paircat(pair):
        return np.ascontiguousarray(np.concatenate(pair, axis=0))

    wls = _paircat(_fp8_pair(len_short_w))
    wll = _paircat(_fp8_pair(len_long_w))
    wsem = [_paircat(_fp8_pair_il(sem_w[e])) for e in range(3)]
    wsyn = [_paircat(_fp8_pair(syn_w_f[e])) for e in range(3)]
    wcls = np.ascontiguousarray(cls_w.astype(_BF16))
    iotaw = np.ascontiguousarray(
        np.arange(S, dtype=np.int32).reshape(S // 16, 16).T)

    in_maps = []
    for b in range(B):
        lencol = 3 if is_short[b] else 4
        rw7 = np.ascontiguousarray(np.concatenate(
            [router_w[:, 0:3], router_w[:, lencol : lencol + 1], router_w[:, 5:8]],
            axis=1, dtype=np.float32))
        wlen = wls if is_short[b] else wll
        hsb = hs[b]
        hs8 = hsb.astype(_F8)
        hsr = (hsb - hs8.astype(np.float32)).astype(_F8)
        hb1 = hsb.astype(_BF16)
        r = hsb - hb1.astype(np.float32)
        hb2 = r.astype(_BF16)
        hb3 = (r - hb2.astype(np.float32)).astype(_BF16)
        rw1 = rw7.astype(_BF16)
        rw2 = (rw7 - rw1.astype(np.float32)).astype(_BF16)
        deg = np.clip(adj[b].sum(axis=1, keepdims=True), 1e-9, None)
        adjq = (ASC * adj[b] / deg).astype(_F8)
        hbm = np.empty((KT, 3, P, S), dtype=_BF16)
        for t, hb in enumerate((hb1.T, hb2.T, hb3.T)):
            hbm[:, t] = hb.reshape(KT, P, S)
        m = {
            "hsb": np.ascontiguousarray(hb1),
            "hbm": np.ascontiguousarray(hbm.reshape(KT * 3 * P, S)),
            "hs8T": np.ascontiguousarray(hs8.T),
            "hsrT": np.ascontiguousarray(hsr.T),
            "hs8r": np.ascontiguousarray(hs8),
            "hsrr": np.ascontiguousarray(hsr),
            "adjT": np.ascontiguousarray(adjq.T),
            "rw1": np.ascontiguousarray(rw1),
            "rw2": np.ascontiguousarray(rw2),
            "wg1": wg1_8, "wg2": wg2_8,
            "wlenp": wlen,
            "wcls": wcls,
            "iotaw": iotaw,
        }
        for e in range(3):
            m[f"wsem{e}p"] = wsem[e]
            m[f"wsyn{e}p"] = wsyn[e]
        if cfg[0]:
            br7 = np.concatenate(
                [router_b[0:3], router_b[lencol : lencol + 1], router_b[5:8]])
            m["br"] = br7.reshape(1, 7).astype(np.float32)
        if cfg[1]:
            m["bsyn"] = (WS * syn_b_f).astype(np.float32)
        if cfg[2]:
            m["blen"] = (WS * (len_short_b if is_short[b]
                               else len_long_b)).reshape(1, H).astype(np.float32)
        if cfg[3]:
            m["bsem"] = (WS * sem_b).astype(np.float32)
        if cfg[4]:
            m["bcls"] = cls_b.reshape(1, 2).astype(np.float32)
        in_maps.append(m)

    try:
        res = bass_utils.run_bass_kernel_spmd(nc, in_maps, core_ids=list(range(B)))
    except Exception:
        # transient device wedge (NRT_EXEC_UNIT_UNRECOVERABLE) clears on retry
        res = bass_utils.run_bass_kernel_spmd(nc, in_maps, core_ids=list(range(B)))
    globals()["_last_results"] = res

    out = np.empty((B, S, 2), dtype=np.float32)
    for b in range(B):
        r = res.results[b]
        outC = np.asarray(r["outC"], dtype=np.float32).transpose(1, 0, 2).reshape(S, 2)
        outB = np.asarray(r["outB"], dtype=np.float32).transpose(1, 0, 2).reshape(NSLOT, 2)
        outA = np.asarray(r["outA"], dtype=np.float32).transpose(1, 0, 2).reshape(NSLOT, 2)
        cnts = np.asarray(r["cnts"]).ravel()
        acc = outC.copy()
        for gi, outX, idx_name in ((0, outA, "idxsyn"), (1, outB, "idxsem")):
            idxw = np.asarray(r[idx_name])
            idx_un = idxw.T.reshape(-1).astype(np.int64)  # slot i = f*16 + q
            for c in range(3):
                nlive = min(int(cnts[3 * gi + c]), CAP)
                slots = np.arange(c * CAP, c * CAP + nlive)
                acc[idx_un[slots]] += outX[slots]
        out[b] = acc
    return out


# revision 34
# speedup vs baseline: 1.0535x; 1.0008x over previous
"""Trainium2 Bass kernel for nn_MoEDetector (moe_routing).

Strategy: data-parallel over batch B=8 -> one batch per NeuronCore, with
top-1 sparse expert dispatch on-device:
  - router logits in fp32 (argmax-selection safe), group softmax ratios
  - GCN chain in single-term fp8 (output is ~5e-4 of the residual stream)
  - tokens are counting-sorted by their syn/sem argmax class on device
    (sparse_gather) into 3 fixed 512-token capacity regions per group;
    dma_gather (indirect DMA) fetches each region's tokens from DRAM in
    transposed [h, slot] layout at zero PE cost
  - each region statically maps to one expert, so the region matmuls run
    at full fp8 DoubleRow speed with 3-term splits (X@W ~ X8@W8 + Xr@W8 +
    X8@Wr); 24 region-tile evals replace 48 dense sem+syn evals
  - the len expert choice is forced per-batch by seq_lengths via router
    masking, so it runs dense in original token order
  - cls head is linear: three partial outputs (len / sem / syn order) are
    produced separately and summed on host after unpermuting
  - expert weights and router hb-splits stream through SBUF in 512-column
    chunks (n-outer loops) to fit the gathered tensors in SBUF
Host-side prep (layout/quantization only; all model FLOPs stay on device):
  - adjacency degree-normalize + fp8 quantize + transpose; hs fp8 pairs in
    both [h,s] and [s,h] layouts; bf16 triple split of hs for the router
  - expert weights as scaled fp8 pairs; sem/syn pairs row-interleaved to
    match dma_gather's u16-granularity transpose of fp8 data
  - LN gain/bias folded into the syn expert weights
"""

import numpy as np
import ml_dtypes
from contextlib import ExitStack

B, S, H = 8, 1024, 1536
THRESHOLD = 128
P = 128
ST = S // P          # 8 s-tiles
KT = H // P          # 12 h contraction tiles
TT = S // P          # 8 t-tiles for adjacency contraction
NCH = 512            # adjacency-matmul moving free-dim chunk
NN = H // NCH        # 3 chunks of the H output dim
WCH = 512            # weight-streamed matmul chunk (S1/S2/len/regions)
WNN = H // WCH       # 3 chunks
KD = KT // 2         # 6 DoubleRow passes over H
TD = TT // 2         # 4 DoubleRow passes over S
WS = 32.0            # host-side weight scale for fp8 range
ASC = 256.0          # host-side adjacency scale for fp8 range
EPS = 1e-5
CAP = 512            # per-class token capacity (tokens per region)
RT = CAP // P        # 4 tiles per region
NSLOT = 3 * CAP      # slots per expert group
GT = NSLOT // P      # 12 slot-tiles per group
NW = NSLOT // 16     # idx columns (wrapped in 16 partitions)
CW = NW // 3         # idx columns per region

_BF16 = ml_dtypes.bfloat16
_F8 = ml_dtypes.float8_e4m3

_prog_cache = {}


def _build_program(cfg):
    """cfg = (router_bias_nz, syn_bias_nz, len_bias_nz, sem_bias_nz, cls_bias_nz)"""
    import concourse.bass as bass
    import concourse.tile as tile
    from concourse import bacc, mybir

    rb_nz, synb_nz, lenb_nz, semb_nz, clsb_nz = cfg
    f32 = mybir.dt.float32
    i32 = mybir.dt.int32
    i16 = mybir.dt.int16
    u32 = mybir.dt.uint32
    bf16 = mybir.dt.bfloat16
    fp8 = mybir.dt.float8e4
    AF = mybir.ActivationFunctionType
    ALU = mybir.AluOpType
    AX = mybir.AxisListType
    DR = mybir.MatmulPerfMode.DoubleRow
    ts = bass.ts

    nc = bacc.Bacc("TRN2", target_bir_lowering=False, debug=False,
                   dynamic_dma_scratch_size=24576)

    # ---- DRAM I/O ----
    hsb_d = nc.dram_tensor("hsb", [S, H], bf16, kind="ExternalInput").ap()
    hbm_d = nc.dram_tensor("hbm", [KT * 3 * P, S], bf16, kind="ExternalInput").ap()
    hs8T_d = nc.dram_tensor("hs8T", [H, S], fp8, kind="ExternalInput").ap()
    hsrT_d = nc.dram_tensor("hsrT", [H, S], fp8, kind="ExternalInput").ap()
    hs8r_d = nc.dram_tensor("hs8r", [S, H], fp8, kind="ExternalInput").ap()
    hsrr_d = nc.dram_tensor("hsrr", [S, H], fp8, kind="ExternalInput").ap()
    adjT_d = nc.dram_tensor("adjT", [S, S], fp8, kind="ExternalInput").ap()
    rw1_d = nc.dram_tensor("rw1", [H, 7], bf16, kind="ExternalInput").ap()
    rw2_d = nc.dram_tensor("rw2", [H, 7], bf16, kind="ExternalInput").ap()
    wg1_d = nc.dram_tensor("wg1", [H, H], fp8, kind="ExternalInput").ap()
    wg2_d = nc.dram_tensor("wg2", [H, H], fp8, kind="ExternalInput").ap()
    iotaw_d = nc.dram_tensor("iotaw", [16, S // 16], i32, kind="ExternalInput").ap()
    wexp_d = []  # merged (w8; wr) rows per expert: len, sem0-2, syn0-2
    for nm in ["len", "sem0", "sem1", "sem2", "syn0", "syn1", "syn2"]:
        wexp_d.append(
            nc.dram_tensor(f"w{nm}p", [2 * H, H], fp8, kind="ExternalInput").ap())
    wcls_d = nc.dram_tensor("wcls", [H, 2], bf16, kind="ExternalInput").ap()
    br_d = nc.dram_tensor("br", [1, 7], f32, kind="ExternalInput").ap() if rb_nz else None
    bsyn_d = nc.dram_tensor("bsyn", [3, H], f32, kind="ExternalInput").ap() if synb_nz else None
    blen_d = nc.dram_tensor("blen", [1, H], f32, kind="ExternalInput").ap() if lenb_nz else None
    bsem_d = nc.dram_tensor("bsem", [3, H], f32, kind="ExternalInput").ap() if semb_nz else None
    bcls_d = nc.dram_tensor("bcls", [1, 2], f32, kind="ExternalInput").ap() if clsb_nz else None
    outC_d = nc.dram_tensor("outC", [P, ST, 2], f32, kind="ExternalOutput").ap()
    outB_d = nc.dram_tensor("outB", [P, GT, 2], f32, kind="ExternalOutput").ap()
    outA_d = nc.dram_tensor("outA", [P, GT, 2], f32, kind="ExternalOutput").ap()
    idxsyn_d = nc.dram_tensor("idxsyn", [16, NW], i16, kind="ExternalOutput").ap()
    idxsem_d = nc.dram_tensor("idxsem", [16, NW], i16, kind="ExternalOutput").ap()
    cnts_d = nc.dram_tensor("cnts", [1, 8], u32, kind="ExternalOutput").ap()

    hs_r = hsb_d.rearrange("(a p) h -> p a h", p=P)
    hbm_r = hbm_d.rearrange("(k t p) s -> p k t s", p=P, t=3)
    hs8T_r = hs8T_d.rearrange("(k p) s -> p k s", p=P)
    hsrT_r = hsrT_d.rearrange("(k p) s -> p k s", p=P)
    adjT_r = adjT_d.rearrange("(t p) s -> p t s", p=P)
    rw1_r = rw1_d.rearrange("(k p) e -> p k e", p=P)
    rw2_r = rw2_d.rearrange("(k p) e -> p k e", p=P)
    wcls_r = wcls_d.rearrange("(k p) c -> p k c", p=P)
    wg1_r = wg1_d.rearrange("(k p) d -> p k d", p=P)
    wg2_r = wg2_d.rearrange("(k p) d -> p k d", p=P)
    wexp_r = [w.rearrange("(k p) d -> p k d", p=P) for w in wexp_d]

    with tile.TileContext(nc) as tc, ExitStack() as ctx:
        # ---- pools ----
        const = ctx.enter_context(tc.tile_pool(name="const", bufs=1))
        hspool = ctx.enter_context(tc.tile_pool(name="hspool", bufs=1))
        f8pool = ctx.enter_context(tc.tile_pool(name="f8pool", bufs=1))
        wpool = ctx.enter_context(tc.tile_pool(name="wpool", bufs=3))
        fpool = ctx.enter_context(tc.tile_pool(name="fpool", bufs=2))
        gpool = ctx.enter_context(tc.tile_pool(name="gpool", bufs=1))
        stage = ctx.enter_context(tc.tile_pool(name="stage", bufs=2))
        small = ctx.enter_context(tc.tile_pool(name="small", bufs=2))
        dram = ctx.enter_context(tc.tile_pool(name="dram", bufs=1, space="DRAM"))
        acc = ctx.enter_context(tc.tile_pool(name="acc", bufs=5, space="PSUM"))
        spsum = ctx.enter_context(tc.tile_pool(name="spsum", bufs=1, space="PSUM"))

        # ---- DRAM scratch (dependency-tracked pool tiles) ----
        shs = dram.tile([S, H], bf16, tag="shs")
        ctab = dram.tile([S, 64], f32, tag="ctab")
        shs_r = shs[:].rearrange("(a p) h -> p a h", p=P)

        # ---- constants (gpsimd DMA queue, parallel to sync queue) ----
        rw1_sb = const.tile([P, KT, 7], bf16, tag="rw1")
        nc.gpsimd.dma_start(rw1_sb[:], rw1_r)
        rw2_sb = const.tile([P, KT, 7], bf16, tag="rw2")
        nc.gpsimd.dma_start(rw2_sb[:], rw2_r)
        wcls_sb = const.tile([P, KT, 2], bf16, tag="wcls")
        nc.gpsimd.dma_start(wcls_sb[:], wcls_r)
        iotaw = const.tile([16, S // 16], i32, tag="iotaw")
        nc.gpsimd.dma_start(iotaw[:], iotaw_d)
        ones_row = None
        if any(x is not None for x in (br_d, bsyn_d, blen_d, bsem_d, bcls_d)):
            ones_row = const.tile([1, P], f32, tag="ones")
            nc.vector.memset(ones_row[:], 1.0)

        def bias_row(dram_ap, n, tag):
            t = const.tile([1, n], f32, tag=tag)
            nc.gpsimd.dma_start(t[:], dram_ap)
            return t

        br_sb = bias_row(br_d, 7, "br") if br_d is not None else None
        blen_sb = bias_row(blen_d, H, "blen") if blen_d is not None else None
        bsem_sb = ([bias_row(bsem_d[e : e + 1, :], H, f"bsem{e}") for e in range(3)]
                   if bsem_d is not None else None)
        bsyn_sb = ([bias_row(bsyn_d[e : e + 1, :], H, f"bsyn{e}") for e in range(3)]
                   if bsyn_d is not None else None)

        # ---- persistent SBUF tensors ----
        hs_all = hspool.tile([P, ST, H], bf16, tag="hs")      # hs -> resid -> fusedC
        hs8T = f8pool.tile([P, KT, S], fp8, tag="hs8T")
        hsrT = f8pool.tile([P, KT, S], fp8, tag="hsrT")
        adjT = f8pool.tile([P, TT, S], fp8, tag="adjT")       # 256 * Anorm^T

        # ---- weight chunk streaming ([P, KT, NCH] slices, consumption order) --
        # all wpool tiles (weight chunks + router hb slices) alternate between
        # the two rotating tag slots in issue order == consumption order
        _tag_ctr = [0]

        def next_tag():
            _tag_ctr[0] += 1
            return "w8" if _tag_ctr[0] % 2 else "wr"

        def load_wc(wdram_r, n, tag=None):
            # GCN single chunks ride the small rotating tags
            wt = wpool.tile([P, KT, WCH], fp8, tag=next_tag(), name=f"wc{_tag_ctr[0]}", bufs=2)
            nc.sync.dma_start(wt[:], wdram_r[:, :, ts(n, WCH)])
            return wt

        def load_wpair(wdram_r, n):
            # merged (w8; wr) chunk: one DMA per pass
            _tag_ctr[0] += 1
            wt = wpool.tile([P, 2 * KT, WCH], fp8, tag="wc", name=f"wp{_tag_ctr[0]}", bufs=2)
            nc.sync.dma_start(wt[:], wdram_r[:, :, ts(n, WCH)])
            return wt

        # router hb k-slices stream through a small pool; DMAs are interleaved
        # with the GCN chunk loads and consumed by router k-blocks interleaved
        # with the GCN matmul phases (so neither queue stalls the other)
        hb_sl = [None] * KT

        def load_hb(k):
            sl = wpool.tile([P, 3, S], bf16, tag=next_tag(), name=f"hb_{k}", bufs=2)
            nc.sync.dma_start(sl[:], hbm_r[:, k, :, :])
            hb_sl[k] = sl

        # GCN1 chunk 0 + full hs8T first (needed for the first psum group)
        nc.sync.dma_start(hs8T[:, 0:6, :], hs8T_r[:, 0:6, :])
        wg1_c0 = load_wc(wg1_r, 0)
        nc.sync.dma_start(hs8T[:, 6:12, :], hs8T_r[:, 6:12, :])
        wg1_c = [wg1_c0] + [load_wc(wg1_r, n) for n in range(1, WNN)]
        nc.sync.dma_start(adjT[:], adjT_r)
        wg2_c = [load_wc(wg2_r, n) for n in range(WNN)]
        for m in range(3):
            nc.sync.dma_start(hs_all[:, m, :], hs_r[:, m, :])
        load_hb(0)
        load_hb(1)
        load_hb(2)
        load_hb(3)
        nc.sync.dma_start(hs_all[:, 3, :], hs_r[:, 3, :])
        load_hb(4)
        load_hb(5)
        nc.sync.dma_start(hs_all[:, 4, :], hs_r[:, 4, :])
        load_hb(6)
        load_hb(7)
        nc.sync.dma_start(hs_all[:, 5, :], hs_r[:, 5, :])
        load_hb(8)
        load_hb(9)
        nc.sync.dma_start(hs_all[:, 6, :], hs_r[:, 6, :])
        load_hb(10)
        load_hb(11)
        nc.sync.dma_start(hs_all[:, 7, :], hs_r[:, 7, :])
        nc.sync.dma_start(hsrT[:], hsrT_r)
        # expert weight chunks: len, sem0-2, syn0-2; (w8, wr) pairs per n-chunk
        wexp_c = []
        for ei in range(7):
            wexp_c.append([load_wpair(wexp_r[ei], n) for n in range(WNN)])

        # ---- router k-blocks (fp32-exact logits from bf16 triple/pair split) --
        logit = small.tile([P, ST, 7], f32, tag="logit", bufs=1)
        nc.vector.memset(logit[:], 0.0)

        def router_k(k):
            rlog = spsum.tile([P, ST, 7], f32, tag="sp")
            hb = hb_sl[k]
            terms = ((0, rw1_sb), (0, rw2_sb), (1, rw1_sb), (1, rw2_sb),
                     (2, rw1_sb))
            for m in range(ST):
                for t_i, (tt, rwt) in enumerate(terms):
                    nc.tensor.matmul(rlog[:, m, :], hb[:, tt, ts(m, P)],
                                     rwt[:, k, :], start=(t_i == 0),
                                     stop=(t_i == len(terms) - 1))
            nc.vector.tensor_add(logit[:], logit[:], rlog[:])

        # ---- GCN: S1 = hs8 @ W1q (fp8 DR), evict /32 -> fp8 [s, d] ----
        s_sb = f8pool.tile([P, ST, H], fp8, tag="s12")
        for n in range(WNN):
            for m in range(ST):
                ps = acc.tile([P, WCH], f32, tag="acc")
                for j in range(KD):
                    nc.tensor.matmul(ps[:], hs8T[:, 2 * j : 2 * j + 2, ts(m, P)],
                                     wg1_c[n][:, 2 * j : 2 * j + 2, :],
                                     start=(j == 0), stop=(j == KD - 1), perf_mode=DR)
                nc.scalar.activation(s_sb[:, m, ts(n, WCH)], ps[:], AF.Copy,
                                     scale=1.0 / WS)

        # ---- x1T = relu(Anorm @ S1)^T via lhsT=S1: store 32*relu(x1) fp8 ----
        x1T = f8pool.tile([P, KT, S], fp8, tag="x1T")
        for dt_i in range(KT):
            for sc in range(2):
                ps = acc.tile([P, NCH], f32, tag="acc")
                for j in range(TD):
                    nc.tensor.matmul(ps[:], s_sb[:, 2 * j : 2 * j + 2, ts(dt_i, P)],
                                     adjT[:, 2 * j : 2 * j + 2, ts(sc, NCH)],
                                     start=(j == 0), stop=(j == TD - 1), perf_mode=DR)
                nc.scalar.activation(x1T[:, dt_i, ts(sc, NCH)], ps[:], AF.Relu,
                                     scale=WS / ASC)

        # ---- S2 = (32 x1) @ W2q: store 32*S2 in fp8 ----
        s2_sb = f8pool.tile([P, ST, H], fp8, tag="s12")
        for n in range(WNN):
            for m in range(ST):
                ps = acc.tile([P, WCH], f32, tag="acc")
                for j in range(KD):
                    nc.tensor.matmul(ps[:], x1T[:, 2 * j : 2 * j + 2, ts(m, P)],
                                     wg2_c[n][:, 2 * j : 2 * j + 2, :],
                                     start=(j == 0), stop=(j == KD - 1), perf_mode=DR)
                nc.scalar.activation(s2_sb[:, m, ts(n, WCH)], ps[:], AF.Copy,
                                     scale=1.0 / WS)

        # ---- residual + LayerNorm -> sh (bf16) ----
        sh_t = [None] * ST

        def do_ln(m):
            stats = small.tile([P, NN, 6], bf16, tag="stats", bufs=1)
            for c in range(NN):
                nc.vector.bn_stats(stats[:, c, :], hs_all[:, m, ts(c, NCH)])
            mv = small.tile([P, 2], f32, tag="mv", bufs=1)
            nc.vector.bn_aggr(mv[:], stats[:])
            # rsqrt via bit-trick seed + Newton step on DVE (keeps Sqrt off Act)
            veps = small.tile([P, 1], f32, tag="veps", bufs=1)
            nc.vector.tensor_scalar(out=veps[:], in0=mv[:, 1:2], scalar1=EPS,
                                    scalar2=None, op0=ALU.add)
            rsd_i = small.tile([P, 1], i32, tag="rsdi", bufs=1)
            nc.vector.tensor_scalar(out=rsd_i[:], in0=veps[:].bitcast(i32),
                                    scalar1=1, scalar2=None,
                                    op0=ALU.logical_shift_right)
            nc.vector.tensor_scalar(out=rsd_i[:], in0=rsd_i[:], scalar1=-1,
                                    scalar2=0x5F3759DF, op0=ALU.mult, op1=ALU.add)
            rstd = rsd_i[:].bitcast(f32)
            nwt = small.tile([P, 1], f32, tag="nwt", bufs=1)
            nc.vector.tensor_mul(nwt[:], rstd, rstd)
            nc.vector.tensor_mul(nwt[:], nwt[:], veps[:])
            nc.vector.tensor_scalar(out=nwt[:], in0=nwt[:], scalar1=-0.5,
                                    scalar2=1.5, op0=ALU.mult, op1=ALU.add)
            nc.vector.tensor_mul(rstd, rstd, nwt[:])
            nmr = small.tile([P, 1], f32, tag="nmr", bufs=1)
            nc.vector.tensor_mul(nmr[:], mv[:, 0:1], rstd)
            nc.vector.tensor_scalar_mul(nmr[:], nmr[:], -1.0)
            sh = stage.tile([P, H], bf16, tag="shm", bufs=2)
            nc.scalar.activation(sh[:], hs_all[:, m, :], AF.Identity,
                                 bias=nmr[:], scale=rstd)
            nc.gpsimd.dma_start(shs_r[:, m, :], sh[:])
            sh_t[m] = sh

        # ---- x2: resid += relu(psum)/8192 (residual adds on DVE) ----
        for m in range(ST):
            for n in range(NN):
                ps = acc.tile([P, NCH], f32, tag="acc")
                for j in range(TD):
                    nc.tensor.matmul(ps[:], adjT[:, 2 * j : 2 * j + 2, ts(m, P)],
                                     s2_sb[:, 2 * j : 2 * j + 2, ts(n, NCH)],
                                     start=(j == 0), stop=(j == TD - 1), perf_mode=DR)
                g = stage.tile([P, NCH], bf16, tag="hTf", bufs=3)
                nc.scalar.activation(g[:], ps[:], AF.Relu, scale=1.0 / (ASC * WS))
                nc.vector.tensor_add(hs_all[:, m, ts(n, NCH)],
                                     hs_all[:, m, ts(n, NCH)], g[:])
                gidx = m * NN + n
                if gidx % 2 == 0 and gidx // 2 < KT:
                    router_k(gidx // 2)
                if n == NN - 1:
                    do_ln(m)

        if br_sb is not None:
            rlog = spsum.tile([P, ST, 7], f32, tag="sp")
            for m in range(ST):
                nc.tensor.matmul(rlog[:, m, :], ones_row[:], br_sb[:],
                                 start=True, stop=True)
            nc.vector.tensor_add(logit[:], logit[:], rlog[:])

        # ---- router math: group softmax ratios + top-1 coefficients ----
        # coef table layout (64 f32 per token): 0=clen, 1..3=csyn, 4..6=csem
        coef_sb = small.tile([P, ST, 64], f32, tag="ctabs", bufs=1)
        nc.vector.memset(coef_sb[:], 0.0)
        e_sb = small.tile([P, ST, 7], f32, tag="esb", bufs=1)
        nc.scalar.activation(e_sb[:], logit[:], AF.Exp)
        gdum = small.tile([1, 4], f32, tag="gdum", bufs=1)
        nc.scalar.activation(gdum[:], e_sb[0:1, 0, 0:4], AF.Gelu)
        syn_e = small.tile([P, ST], f32, tag="syn_e", bufs=1)
        nc.vector.tensor_reduce(syn_e[:], e_sb[:, :, 0:3], axis=AX.X, op=ALU.max)
        sem_e = small.tile([P, ST], f32, tag="sem_e", bufs=1)
        nc.vector.tensor_reduce(sem_e[:], e_sb[:, :, 4:7], axis=AX.X, op=ALU.max)
        rden = small.tile([P, ST], f32, tag="rden", bufs=1)
        nc.vector.tensor_add(rden[:], syn_e[:], sem_e[:])
        nc.vector.tensor_add(rden[:], rden[:], e_sb[:, :, 3])
        nc.vector.reciprocal(rden[:], rden[:])
        nc.vector.tensor_mul(coef_sb[:, :, 0], e_sb[:, :, 3], rden[:])

        cls_f = [None, None]  # f32 class vecs: [syn, sem]

        def group_coefs(gi, ccol, base, w_e):
            """coef cols ccol..ccol+2 = rden * w_e * mask_e; class vec = first-max
            argmax over logit columns base..base+2 (matches jnp tie-breaking)."""
            l0, l1, l2 = (logit[:, :, base + i] for i in range(3))
            s0 = small.tile([P, ST], f32, tag="s0", bufs=1)
            ge02 = small.tile([P, ST], f32, tag="ge02", bufs=1)
            nc.vector.tensor_tensor(out=s0[:], in0=l0, in1=l1, op=ALU.is_ge)
            nc.vector.tensor_tensor(out=ge02[:], in0=l0, in1=l2, op=ALU.is_ge)
            nc.vector.tensor_mul(s0[:], s0[:], ge02[:])
            s1 = small.tile([P, ST], f32, tag="s1", bufs=1)
            ge12 = small.tile([P, ST], f32, tag="ge12", bufs=1)
            nc.vector.tensor_tensor(out=ge12[:], in0=l1, in1=l2, op=ALU.is_ge)
            nc.vector.tensor_mul(s1[:], s0[:], ge12[:])
            nc.vector.tensor_tensor(out=s1[:], in0=ge12[:], in1=s1[:], op=ALU.subtract)
            s2 = small.tile([P, ST], f32, tag="s2", bufs=1)
            nc.vector.tensor_add(s2[:], s0[:], s1[:])
            nc.vector.tensor_scalar(out=s2[:], in0=s2[:], scalar1=-1.0, scalar2=1.0,
                                    op0=ALU.mult, op1=ALU.add)
            for e, sm in enumerate((s0, s1, s2)):
                nc.vector.tensor_mul(coef_sb[:, :, ccol + e], sm[:], w_e)
                nc.vector.tensor_mul(coef_sb[:, :, ccol + e],
                                     coef_sb[:, :, ccol + e], rden[:])
            cg = small.tile([P, ST], f32, tag=f"clsv{gi}", bufs=1)
            nc.vector.tensor_scalar(out=cg[:], in0=s2[:], scalar1=2.0, scalar2=None,
                                    op0=ALU.mult)
            nc.vector.tensor_add(cg[:], cg[:], s1[:])
            cls_f[gi] = cg

        group_coefs(0, 1, 0, syn_e[:])
        group_coefs(1, 4, 4, sem_e[:])
        nc.gpsimd.dma_start(ctab[:].rearrange("(a p) c -> p a c", p=P), coef_sb[:])

        # ---- counting sort per group: wrapped class -> sparse_gather lists ----
        cnts_sb = small.tile([1, 8], u32, tag="cnts", bufs=1)
        nc.vector.memset(cnts_sb[:], 0)
        idx_tiles = []
        for gi in range(2):  # 0=syn, 1=sem
            cg_i = small.tile([P, ST], i32, tag="cgi", bufs=1)
            nc.vector.tensor_copy(cg_i[:], cls_f[gi][:])
            clsw = small.tile([16, ST, 8], i32, tag="clsw", bufs=1)
            for r in range(8):
                nc.gpsimd.dma_start(clsw[:, :, r], cg_i[r * 16:(r + 1) * 16, :])
            clsw_f = clsw[:].rearrange("q a r -> q (a r)")
            arr = small.tile([16, 3, S // 16], f32, tag="arr", bufs=1)
            msk = small.tile([16, S // 16], i32, tag="msk", bufs=1)
            iop = small.tile([16, S // 16], i32, tag="iop", bufs=1)
            for c in range(3):
                nc.vector.tensor_scalar(out=msk[:], in0=clsw_f, scalar1=c,
                                        scalar2=None, op0=ALU.is_equal)
                nc.vector.tensor_scalar(out=iop[:], in0=iotaw[:], scalar1=1,
                                        scalar2=None, op0=ALU.add)
                nc.vector.tensor_tensor(out=iop[:], in0=msk[:], in1=iop[:],
                                        op=ALU.mult)
                nc.vector.tensor_scalar(out=arr[:, c, :], in0=iop[:], scalar1=-1,
                                        scalar2=None, op0=ALU.add)
            glist = small.tile([16, 3, S // 16], f32, tag="gl", bufs=1)
            for c in range(3):
                nc.gpsimd.sparse_gather(
                    glist[:, c, :], arr[:, c, :],
                    num_found=cnts_sb[0:1, 3 * gi + c : 3 * gi + c + 1])
            gmax = small.tile([16, 3, CW], f32, tag="gmax", bufs=1)
            nc.vector.tensor_scalar(out=gmax[:], in0=glist[:, :, 0:CW],
                                    scalar1=0.0, scalar2=float(S - 1),
                                    op0=ALU.max, op1=ALU.min)
            idxs = gpool.tile([P, NW], i16, tag=f"idx{gi}", name=f"idx{gi}")
            nc.vector.tensor_copy(idxs[0:16, :], gmax[:].rearrange("q c n -> q (c n)"))
            for g in range(1, 8):
                nc.gpsimd.dma_start(idxs[g * 16:(g + 1) * 16, :], idxs[0:16, :])
            idx_tiles.append(idxs)
            nc.sync.dma_start(idxsyn_d if gi == 0 else idxsem_d, idxs[0:16, :])
        nc.sync.dma_start(cnts_d, cnts_sb[:])
        idx_syn, idx_sem = idx_tiles

        # ---- indirect gathers: region x tensors + coef tables ----
        # gathered layout per region: [p, f, s, b] with h = 256f + 2p + b
        def gather_x(src_dram, idxs, tags, pool):
            tiles = []
            for c in range(3):
                t = pool.tile([P, 6, CAP, 2], fp8, tag=tags[c], name=f"{tags[c]}x")
                gv = t[:].rearrange("p f s b -> p (f s b)").rearrange(
                    "p (t n) -> p t n", t=KT)
                nc.gpsimd.dma_gather(gv, src_dram, idxs[:, c * CW:(c + 1) * CW],
                                     CAP, CAP, H, elem_step=H, transpose=True)
                tiles.append(t)
            return tiles

        def gather_coef(idxs, tag):
            t = gpool.tile([P, GT, 8], f32, tag=tag, name=tag)
            for c in range(3):
                cscr = gpool.tile([P, RT, 64], f32, tag="cscr", name=f"cs{tag}{c}")
                nc.gpsimd.dma_gather(cscr[:], ctab[:],
                                     idxs[:, c * CW:(c + 1) * CW],
                                     CAP, CAP, 64, elem_step=64, transpose=False)
                nc.vector.tensor_copy(t[:, c * RT:(c + 1) * RT, :],
                                      cscr[:, :, 0:8])
            return t

        xg8_sem = gather_x(hs8r_d, idx_sem, ["s12", "x1T", "adjT"], f8pool)
        xgr_sem = gather_x(hsrr_d, idx_sem, ["xgb0", "xgb1", "xgb2"], gpool)
        coefp_sem = gather_coef(idx_sem, "cpsem")
        coefp_syn = gather_coef(idx_syn, "cpsyn")


        # ---- len expert (dense, original token order) into hs_all ----

        x8, xr = hs8T, hsrT
        for n in range(WNN):
            wp = wexp_c[0][n]
            for m in range(ST):
                ps = acc.tile([P, WCH], f32, tag="acc")
                for t_i, (xx, ko) in enumerate(((x8, 0), (xr, 0), (x8, KT))):
                    for j in range(KD):
                        last = (t_i == 2 and j == KD - 1 and blen_sb is None)
                        nc.tensor.matmul(
                            ps[:], xx[:, 2 * j : 2 * j + 2, ts(m, P)],
                            wp[:, ko + 2 * j : ko + 2 * j + 2, :],
                            start=(t_i == 0 and j == 0), stop=last, perf_mode=DR)
                if blen_sb is not None:
                    nc.tensor.matmul(ps[:], ones_row[:], blen_sb[:, ts(n, WCH)],
                                     start=False, stop=True)
                g = stage.tile([P, WCH], bf16, tag="hTf", bufs=3)
                nc.scalar.activation(g[:], ps[:], AF.Gelu, scale=1.0 / WS)
                nc.vector.tensor_scalar_mul(hs_all[:, m, ts(n, WCH)], g[:],
                                            coef_sb[:, m, 0:1])

        # ---- cls tail helpers (linear head applied per partial output) ----
        out_sbC = small.tile([P, ST, 2], f32, tag="outC", bufs=1)
        out_sbB = small.tile([P, GT, 2], f32, tag="outB", bufs=1)
        out_sbA = small.tile([P, GT, 2], f32, tag="outA", bufs=1)
        bcls_sb = bias_row(bcls_d, 2, "bcls") if bcls_d is not None else None

        def make_tail(out_sb, with_bias):
            fuT_t = {}

            def pre(src_ap, m):
                fuT = stage.tile([P, KT, P], bf16, tag="fuT", bufs=4)
                nc.scalar.dma_start_transpose(fuT[:], src_ap)
                fuT_t[m] = fuT

            def cls(m):
                fuT = fuT_t.pop(m)
                cps = spsum.tile([P, 2], f32, tag="cls")
                for k in range(KT):
                    last = (k == KT - 1) and not (with_bias and bcls_sb is not None)
                    nc.tensor.matmul(cps[:], fuT[:, k, :], wcls_sb[:, k, :],
                                     start=(k == 0), stop=last)
                if with_bias and bcls_sb is not None:
                    nc.tensor.matmul(cps[:], ones_row[:], bcls_sb[:],
                                     start=False, stop=True)
                nc.vector.tensor_copy(out_sb[:, m, :], cps[:])

            return pre, cls

        preC, clsC = make_tail(out_sbC, True)
        preB, clsB = make_tail(out_sbB, False)
        preA, clsA = make_tail(out_sbA, False)

        # ---- region experts: region c of a group evaluates expert c ----
        # (per-slot coefs are zero for other-class tokens / dead pad slots)
        def region_expert(xg8, xgr, wcs, coefp, ccol, c, bias_sb, after_tile,
                          il=True):
            fb = fpool.tile([P, RT, H], bf16, tag="fus", name=f"fus{ccol}_{c}")
            for n in range(WNN):
                wp = wcs[n]
                for mm in range(RT):
                    ps = acc.tile([P, WCH], f32, tag="acc")
                    k = 0
                    for xx, ko in ((xg8, 0), (xgr, 0), (xg8, KT)):
                        for jj in range(KD):
                            k += 1
                            last = (k == 18 and bias_sb is None)
                            if il:
                                bb, f0 = jj // 3, 2 * (jj % 3)
                                lhsT = xx[:, f0:f0 + 2, ts(mm, P), bb]
                                rhs = wp[:, ko + 6 * bb + f0 : ko + 6 * bb + f0 + 2, :]
                            else:
                                lhsT = xx[:, 2 * jj:2 * jj + 2, ts(mm, P)]
                                rhs = wp[:, ko + 2 * jj : ko + 2 * jj + 2, :]
                            nc.tensor.matmul(ps[:], lhsT, rhs,
                                             start=(k == 1), stop=last,
                                             perf_mode=DR)
                    if bias_sb is not None:
                        nc.tensor.matmul(ps[:], ones_row[:], bias_sb[:, ts(n, WCH)],
                                         start=False, stop=True)
                    g = stage.tile([P, WCH], bf16, tag="hTf", bufs=3)
                    nc.scalar.activation(g[:], ps[:], AF.Gelu, scale=1.0 / WS)
                    nc.vector.tensor_scalar_mul(
                        fb[:, mm, ts(n, WCH)], g[:],
                        coefp[:, c * RT + mm, ccol:ccol + 1])
                    after_tile(fb, n * RT + mm, c * RT + mm)

        # sem phase: fusedB region tails + fusedC (hs_all) tails, with cls
        # trailing its transpose by two hooks to hide the fuT DMA latency
        # pending cls calls drain in the NEXT region's early steps, giving
        # each fuT transpose a full region of lead time
        pending_cls = []

        def drain_cls(k=1):
            for _ in range(k):
                if pending_cls:
                    pending_cls.pop(0)()

        def sem_after(fb, si, gm, creg):
            if creg == 0 and si < ST:
                preC(hs_all[:, si, :], si)
                pending_cls.append(lambda m=si: clsC(m))
            if si < (WNN - 1) * RT:
                if si >= 4:
                    drain_cls()
                return
            mm = gm % RT
            preB(fb[:, mm, :], gm)
            pending_cls.append(lambda gm=gm: clsB(gm))
            if gm == ST - 1:
                pending_cls.append(
                    lambda: nc.sync.dma_start(outC_d, out_sbC[:]))

        # syn-group gather+split: bf16 transpose-gather of shared rows into a
        # temp (standard k-tile layout), then split to the fp8 pair on-chip;
        # issued between sem regions so each region's WAR deps resolve in turn
        xg8_syn = [None] * 3
        xgr_syn = [None] * 3
        syn_tmp = [None] * 3

        def syn_gather(c):
            tmp = f8pool.tile([P, KT, CAP], bf16, tag="hs8T" if c % 2 == 0 else "hsrT",
                              name=f"sgt{c}")
            nc.gpsimd.dma_gather(tmp[:], shs[:], idx_syn[:, c * CW:(c + 1) * CW],
                                 CAP, CAP, H, elem_step=H, transpose=True)
            syn_tmp[c] = tmp
            x8 = f8pool.tile([P, KT, CAP], fp8, tag=("s12", "x1T", "adjT")[c],
                             name=f"sg8{c}")
            xr = gpool.tile([P, KT, CAP], fp8, tag=f"xgb{c}", name=f"sgr{c}")
            xg8_syn[c] = x8
            xgr_syn[c] = xr

        def split_views(c, h):
            tmp3 = syn_tmp[c][:].rearrange("p k (q n) -> p (k q) n", q=RT)
            x83 = xg8_syn[c][:].rearrange("p k (q n) -> p (k q) n", q=RT)
            xr3 = xgr_syn[c][:].rearrange("p k (q n) -> p (k q) n", q=RT)
            part = slice(h * 4, h * 4 + 4)
            return tmp3[:, part, :], x83[:, part, :], xr3[:, part, :]

        def split_copy(c, h):
            tmp_p, x8_p, _ = split_views(c, h)
            nc.scalar.activation(x8_p, tmp_p, AF.Copy)

        def split_sub(c, h):
            tmp_p, x8_p, xr_p = split_views(c, h)
            nc.vector.tensor_tensor(out=xr_p, in0=tmp_p, in1=x8_p,
                                    op=ALU.subtract)

        for c in range(3):
            # interleave split pieces of region c-1's syn input into this
            # region's eval steps (slots freed at region c-1's end)
            cc = c - 1  # split pieces for the previous region's syn input

            def hook(fb, si, gm, cc=cc):
                sem_after(fb, si, gm, cc + 1)
                if cc < 0:
                    return
                if si < KT:
                    split_copy(cc, si)
                if 2 <= si < KT + 2:
                    split_sub(cc, si - 2)

            region_expert(xg8_sem[c], xgr_sem[c], wexp_c[1 + c], coefp_sem,
                          4 + c, c, bsem_sb[c] if bsem_sb else None, hook)
            syn_gather(c)

        def syn_after(fa, si, gm):
            if si < (WNN - 1) * RT:
                drain_cls()
                return
            mm = gm % RT
            preA(fa[:, mm, :], gm)
            pending_cls.append(lambda gm=gm: clsA(gm))
            if gm == GT - 1:
                pending_cls.append(
                    lambda: nc.sync.dma_start(outB_d, out_sbB[:]))

        for c in range(3):
            def hook(fa, si, gm, c=c):
                syn_after(fa, si, gm)
                if c != 0:
                    return
                if si < KT:
                    split_copy(2, si)
                if 2 <= si < KT + 2:
                    split_sub(2, si - 2)

            region_expert(xg8_syn[c], xgr_syn[c], wexp_c[4 + c], coefp_syn,
                          1 + c, c, bsyn_sb[c] if bsyn_sb else None, hook,
                          il=False)
        drain_cls(len(pending_cls))
        nc.sync.dma_start(outA_d, out_sbA[:])

    nc.compile()
    return nc


def _get_program(cfg):
    if cfg not in _prog_cache:
        _prog_cache[cfg] = _build_program(cfg)
    return _prog_cache[cfg]


def _fp8_pair(w):
    """w -> (q8(32w), q8(32w - float(q8(32w)))) as contiguous fp8 arrays."""
    ws = (WS * w).astype(np.float32)
    w8 = ws.astype(_F8)
    wr = (ws - w8.astype(np.float32)).astype(_F8)
    return np.ascontiguousarray(w8), np.ascontiguousarray(wr)


# dma_gather transposes fp8 at u16 granularity: gathered[p, f, s, b] holds
# h = 256f + 2p + b; sem/syn weight rows are pre-permuted to match, with
# k-tile t = 6b + f containing rows 256f + 2p + b.
_IL_ROWS = np.empty(H, dtype=np.int64)
for _b in range(2):
    for _f in range(6):
        _IL_ROWS[(6 * _b + _f) * P:(6 * _b + _f + 1) * P] = \
            256 * _f + 2 * np.arange(P) + _b


def _fp8_pair_il(w):
    w8, wr = _fp8_pair(w)
    return (np.ascontiguousarray(w8[_IL_ROWS, :]),
            np.ascontiguousarray(wr[_IL_ROWS, :]))


def kernel(**inputs):
    from concourse import bass_utils

    hs = np.asarray(inputs["hidden_states"], dtype=np.float32)
    adj = np.asarray(inputs["adj_matrix"], dtype=np.float32)
    seq_lengths = np.asarray(inputs["seq_lengths"])
    router_w = np.asarray(inputs["router_w"], dtype=np.float32)
    router_b = np.asarray(inputs["router_b"], dtype=np.float32)
    gcn1_w = np.asarray(inputs["gcn1_w"], dtype=np.float32)
    gcn2_w = np.asarray(inputs["gcn2_w"], dtype=np.float32)
    ln_g = np.asarray(inputs["ln_g"], dtype=np.float32)
    ln_b = np.asarray(inputs["ln_b"], dtype=np.float32)
    syn_w = np.asarray(inputs["syn_w"], dtype=np.float32)
    syn_b = np.asarray(inputs["syn_b"], dtype=np.float32)
    len_short_w = np.asarray(inputs["len_short_w"], dtype=np.float32)
    len_short_b = np.asarray(inputs["len_short_b"], dtype=np.float32)
    len_long_w = np.asarray(inputs["len_long_w"], dtype=np.float32)
    len_long_b = np.asarray(inputs["len_long_b"], dtype=np.float32)
    sem_w = np.asarray(inputs["sem_w"], dtype=np.float32)
    sem_b = np.asarray(inputs["sem_b"], dtype=np.float32)
    cls_w = np.asarray(inputs["cls_w"], dtype=np.float32)
    cls_b = np.asarray(inputs["cls_b"], dtype=np.float32)

    # fold LN affine into syn expert weights
    syn_w_f = (ln_g[None, :, None] * syn_w).astype(np.float32)
    syn_b_f = (syn_b + np.einsum("h,ehd->ed", ln_b, syn_w)).astype(np.float32)

    is_short = seq_lengths <= THRESHOLD

    cfg = (
        bool(np.any(router_b != 0)),
        bool(np.any(syn_b_f != 0)),
        bool(np.any(len_short_b != 0) or np.any(len_long_b != 0)),
        bool(np.any(sem_b != 0)),
        bool(np.any(cls_b != 0)),
    )
    nc = _get_program(cfg)

    wg1_8, _ = _fp8_pair(gcn1_w)
    wg2_8, _ = _fp8_pair(gcn2_w)

    def _paircat(pair):
        return np.ascontiguousarray(np.concatenate(pair, axis=0))

    wls = _paircat(_fp8_pair(len_short_w))
    wll = _paircat(_fp8_pair(len_long_w))
    wsem = [_paircat(_fp8_pair_il(sem_w[e])) for e in range(3)]
    wsyn = [_paircat(_fp8_pair(syn_w_f[e])) for e in range(3)]
    wcls = np.ascontiguousarray(cls_w.astype(_BF16))
    iotaw = np.ascontiguousarray(
        np.arange(S, dtype=np.int32).reshape(S // 16, 16).T)

    in_maps = []
    for b in range(B):
        lencol = 3 if is_short[b] else 4
        rw7 = np.ascontiguousarray(np.concatenate(
            [router_w[:, 0:3], router_w[:, lencol : lencol + 1], router_w[:, 5:8]],
            axis=1, dtype=np.float32))
        wlen = wls if is_short[b] else wll
        hsb = hs[b]
        hs8 = hsb.astype(_F8)
        hsr = (hsb - hs8.astype(np.float32)).astype(_F8)
        hb1 = hsb.astype(_BF16)
        r = hsb - hb1.astype(np.float32)
        hb2 = r.astype(_BF16)
        hb3 = (r - hb2.astype(np.float32)).astype(_BF16)
        rw1 = rw7.astype(_BF16)
        rw2 = (rw7 - rw1.astype(np.float32)).astype(_BF16)
        deg = np.clip(adj[b].sum(axis=1, keepdims=True), 1e-9, None)
        adjq = (ASC * adj[b] / deg).astype(_F8)
        hbm = np.empty((KT, 3, P, S), dtype=_BF16)
        for t, hb in enumerate((hb1.T, hb2.T, hb3.T)):
            hbm[:, t] = hb.reshape(KT, P, S)
        m = {
            "hsb": np.ascontiguousarray(hb1),
            "hbm": np.ascontiguousarray(hbm.reshape(KT * 3 * P, S)),
            "hs8T": np.ascontiguousarray(hs8.T),
            "hsrT": np.ascontiguousarray(hsr.T),
            "hs8r": np.ascontiguousarray(hs8),
            "hsrr": np.ascontiguousarray(hsr),
            "adjT": np.ascontiguousarray(adjq.T),
            "rw1": np.ascontiguousarray(rw1),
            "rw2": np.ascontiguousarray(rw2),
            "wg1": wg1_8, "wg2": wg2_8,
            "wlenp": wlen,
            "wcls": wcls,
            "iotaw": iotaw,
        }
        for e in range(3):
            m[f"wsem{e}p"] = wsem[e]
            m[f"wsyn{e}p"] = wsyn[e]
        if cfg[0]:
            br7 = np.concatenate(
                [router_b[0:3], router_b[lencol : lencol + 1], router_b[5:8]])
            m["br"] = br7.reshape(1, 7).astype(np.float32)
        if cfg[1]:
            m["bsyn"] = (WS * syn_b_f).astype(np.float32)
        if cfg[2]:
            m["blen"] = (WS * (len_short_b if is_short[b]
                               else len_long_b)).reshape(1, H).astype(np.float32)
        if cfg[3]:
            m["bsem"] = (WS * sem_b).astype(np.float32)
        if cfg[4]:
            m["bcls"] = cls_b.reshape(1, 2).astype(np.float32)
        in_maps.append(m)

    try:
        res = bass_utils.run_bass_kernel_spmd(nc, in_maps, core_ids=list(range(B)))
    except Exception:
        # transient device wedge (NRT_EXEC_UNIT_UNRECOVERABLE) clears on retry
        res = bass_utils.run_bass_kernel_spmd(nc, in_maps, core_ids=list(range(B)))
    globals()["_last_results"] = res

    out = np.empty((B, S, 2), dtype=np.float32)
    for b in range(B):
        r = res.results[b]
        outC = np.asarray(r["outC"], dtype=np.float32).transpose(1, 0, 2).reshape(S, 2)
        outB = np.asarray(r["outB"], dtype=np.float32).transpose(1, 0, 2).reshape(NSLOT, 2)
        outA = np.asarray(r["outA"], dtype=np.float32).transpose(1, 0, 2).reshape(NSLOT, 2)
        cnts = np.asarray(r["cnts"]).ravel()
        acc = outC.copy()
        for gi, outX, idx_name in ((0, outA, "idxsyn"), (1, outB, "idxsem")):
            idxw = np.asarray(r[idx_name])
            idx_un = idxw.T.reshape(-1).astype(np.int64)  # slot i = f*16 + q
            for c in range(3):
                nlive = min(int(cnts[3 * gi + c]), CAP)
                slots = np.arange(c * CAP, c * CAP + nlive)
                acc[idx_un[slots]] += outX[slots]
        out[b] = acc
    return out
